# revision 5
# baseline (speedup 1.0000x reference)
"""Trainium2 Bass kernel for nn_GCNModelGumbel (gumbel-softmax skip-gram loss).

Math (matching reference.py, with z = stop_gradient(y_hard - y_soft) + y_soft
== y_hard numerically == onehot(argmax(q + gumbel)) for temp > 0):

  q[b]      = (node_emb[w_b] * node_emb[c_b]) @ W^T          [B, 64]
  y         = softmax(q)                                      (output 2)
  prior     = softmax(node_emb[w_b] @ W^T)                    (output 3)
  k*_b      = argmax(q + gumbel)
  loss      = mean_b[ sp(-proj[c_b, k*]) + 0.2 * sum_n sp(proj[neg_bn, k*]) ]
  where proj = ctx_emb @ W^T and sp = softplus                (output 1)

Two SPMD launches on 8 cores:
  L1: each core computes a 1/8 slice of Gp = sp(-proj), Gn2 = 0.2*sp(proj).
  L2: data-parallel over batch; per 2048-batch chunk: indirect row-gathers of
      node_emb for w/c, on-chip matmuls + softmaxes + argmax, then indirect
      element gathers of Gp/Gn2 at flat offsets idx*64 + k*.
"""
import sys
sys.path.insert(0, '/opt/trn_rl_repo')

import numpy as np

import concourse.bacc as bacc
import concourse.bass as bass
import concourse.mybir as mybir
import concourse.tile as tile
from concourse.masks import make_identity

SIZE = 100000
D = 128
K = 64
B = 131072
N_NEG = 5
N_CORES = 8
B_CORE = B // N_CORES            # 16384
CHUNK = 2048                     # batches per chunk
N_CHUNK = B_CORE // CHUNK        # 8
N_SUB = CHUNK // 128             # 16 subtiles per chunk

F32 = mybir.dt.float32
I32 = mybir.dt.int32

_COMPILED = {}


def _run_spmd(nc, in_maps):
    from concourse.bass_utils import run_bass_kernel_spmd
    return run_bass_kernel_spmd(nc, in_maps, core_ids=list(range(N_CORES)))


# --------------------------------------------------------------------------- #
# Launch 1: per-core slice of Gp / Gn2 tables
# --------------------------------------------------------------------------- #

def build_l1(rows):
    """rows = number of ctx_emb rows this core handles (SIZE/8 = 12500)."""
    nc = bacc.Bacc(None, target_bir_lowering=False)
    ctx = nc.dram_tensor('ctx', [rows, D], F32, kind='ExternalInput')
    cw = nc.dram_tensor('cw', [K, D], F32, kind='ExternalInput')
    gp = nc.dram_tensor('gp', [rows, K], F32, kind='ExternalOutput')
    gn = nc.dram_tensor('gn', [rows, K], F32, kind='ExternalOutput')

    n_full = rows // 128
    rem = rows - n_full * 128
    with tile.TileContext(nc) as tc:
        with tc.tile_pool(name='const', bufs=1) as cpool, \
             tc.tile_pool(name='work', bufs=3) as pool, \
             tc.tile_pool(name='ps', bufs=2, space='PSUM') as psum, \
             tc.tile_pool(name='ps2', bufs=2, space='PSUM') as psum2:
            ident = cpool.tile([128, 128], F32)
            make_identity(nc, ident[:])
            cw_t = cpool.tile([K, D], F32)
            nc.sync.dma_start(cw_t[:], cw[:])
            wT_ps = psum.tile([128, K], F32, tag='wT')
            nc.tensor.transpose(out=wT_ps[:], in_=cw_t[:], identity=ident[:K, :K])
            wT = cpool.tile([128, K], F32)
            nc.vector.tensor_copy(wT[:], wT_ps[:])

            for t in range(n_full + (1 if rem else 0)):
                p = 128 if t < n_full else rem
                r0 = t * 128
                rowst = pool.tile([128, D], F32, tag='rows')
                nc.sync.dma_start(rowst[:p, :], ctx[r0:r0 + p, :])
                xT_ps = psum.tile([128, 128], F32, tag='xT')
                nc.tensor.transpose(out=xT_ps[:, :p], in_=rowst[:p, :],
                                    identity=ident[:p, :p])
                xT = pool.tile([128, 128], F32, tag='xTs')
                nc.scalar.copy(xT[:, :p], xT_ps[:, :p])
                pr_ps = psum2.tile([128, K], F32, tag='proj')
                nc.tensor.matmul(pr_ps[:p, :], lhsT=xT[:, :p], rhs=wT[:],
                                 start=True, stop=True)
                # Gp = ln(1 + exp(-proj))
                e1 = pool.tile([128, K], F32, tag='e1')
                nc.scalar.activation(e1[:p, :], pr_ps[:p, :],
                                     mybir.ActivationFunctionType.Exp, scale=-1.0)
                g1 = pool.tile([128, K], F32, tag='g1')
                nc.scalar.activation(g1[:p, :], e1[:p, :],
                                     mybir.ActivationFunctionType.Ln, bias=1.0)
                nc.sync.dma_start(gp[r0:r0 + p, :], g1[:p, :])
                # Gn2 = 0.2 * ln(1 + exp(proj))
                e2 = pool.tile([128, K], F32, tag='e2')
                nc.scalar.activation(e2[:p, :], pr_ps[:p, :],
                                     mybir.ActivationFunctionType.Exp)
                g2 = pool.tile([128, K], F32, tag='g2')
                nc.scalar.activation(g2[:p, :], e2[:p, :],
                                     mybir.ActivationFunctionType.Ln, bias=1.0)
                g2s = pool.tile([128, K], F32, tag='g2s')
                nc.vector.tensor_scalar_mul(g2s[:p, :], g2[:p, :], 1.0 / N_NEG)
                nc.sync.dma_start(gn[r0:r0 + p, :], g2s[:p, :])
    nc.finalize()
    return nc


# --------------------------------------------------------------------------- #
# Launch 2: main kernel (per-core batch shard)
# --------------------------------------------------------------------------- #

def build_l2():
    nc = bacc.Bacc(None, target_bir_lowering=False)
    node = nc.dram_tensor('node', [SIZE, D], F32, kind='ExternalInput')
    gp = nc.dram_tensor('gp', [SIZE, K], F32, kind='ExternalInput')
    gn = nc.dram_tensor('gn', [SIZE, K], F32, kind='ExternalInput')
    cw = nc.dram_tensor('cw', [K, D], F32, kind='ExternalInput')
    # host-relaid index tensors: [nchunk, 128, ...] with batch = base + j*128 + p
    wl = nc.dram_tensor('wl', [N_CHUNK, 128, N_SUB], I32, kind='ExternalInput')
    cl = nc.dram_tensor('cl', [N_CHUNK, 128, N_SUB], I32, kind='ExternalInput')
    negl = nc.dram_tensor('negl', [N_CHUNK, 128, N_SUB, N_NEG], I32,
                          kind='ExternalInput')
    gum = nc.dram_tensor('gum', [B_CORE, K], F32, kind='ExternalInput')
    y_out = nc.dram_tensor('y_out', [B_CORE, K], F32, kind='ExternalOutput')
    p_out = nc.dram_tensor('p_out', [B_CORE, K], F32, kind='ExternalOutput')
    loss_out = nc.dram_tensor('loss_out', [128, 1], F32, kind='ExternalOutput')


    def chunk_view(t, base):
        # [B_CORE, K] dram tensor viewed as [p=128, a=N_SUB, b=K] for batch
        # row = base + a*128 + p
        return bass.AP(t, base * K, [[K, 128], [128 * K, N_SUB], [1, K]])
    gp_flat = gp.rearrange("v k -> (v k) ()")
    gn_flat = gn.rearrange("v k -> (v k) ()")

    with tile.TileContext(nc) as tc:
        with tc.tile_pool(name='const', bufs=1) as cpool, \
             tc.tile_pool(name='io', bufs=2) as iop, \
             tc.tile_pool(name='big', bufs=2) as bigp, \
             tc.tile_pool(name='mid', bufs=2) as midp, \
             tc.tile_pool(name='sm', bufs=3) as smp, \
             tc.tile_pool(name='tp', bufs=4) as tpp, \
             tc.tile_pool(name='psq', bufs=1, space='PSUM') as psq, \
             tc.tile_pool(name='psp', bufs=1, space='PSUM') as psp, \
             tc.tile_pool(name='pst', bufs=2, space='PSUM') as pst:
            ident = cpool.tile([128, 128], F32)
            make_identity(nc, ident[:])
            cw_t = cpool.tile([K, D], F32)
            nc.sync.dma_start(cw_t[:], cw[:])
            wT_ps = pst.tile([128, K], F32, tag='xT')
            nc.tensor.transpose(out=wT_ps[:], in_=cw_t[:], identity=ident[:K, :K])
            wT = cpool.tile([128, K], F32)
            nc.vector.tensor_copy(wT[:], wT_ps[:])
            # iota along K repeated per subtile: [128, N_SUB, K], value = k
            iota_i = cpool.tile([128, N_SUB * K], I32)
            nc.gpsimd.iota(iota_i[:], pattern=[[0, N_SUB], [1, K]],
                           channel_multiplier=0)
            iota_f = cpool.tile([128, N_SUB, K], F32)
            nc.vector.tensor_copy(iota_f[:].rearrange("p a b -> p (a b)"), iota_i[:])
            # reversed iota 64-k (for first-argmax tie-break): value = K - k
            iota_rev = cpool.tile([128, N_SUB, K], F32)
            nc.vector.tensor_scalar(
                out=iota_rev[:].rearrange("p a b -> p (a b)"),
                in0=iota_f[:].rearrange("p a b -> p (a b)"),
                scalar1=-1.0, scalar2=float(K),
                op0=mybir.AluOpType.mult, op1=mybir.AluOpType.add)
            acc = cpool.tile([128, 1], F32)
            nc.vector.memset(acc[:], 0.0)

            for ch in range(N_CHUNK):
                base = ch * CHUNK
                idxw = iop.tile([128, N_SUB], I32, tag='idxw')
                nc.sync.dma_start(idxw[:], wl[ch])
                idxc = iop.tile([128, N_SUB], I32, tag='idxc')
                nc.sync.dma_start(idxc[:], cl[ch])
                idxn = iop.tile([128, N_SUB, N_NEG], I32, tag='idxn')
                nc.sync.dma_start(idxn[:], negl[ch])
                gtile = bigp.tile([128, N_SUB, K], F32, tag='gum')
                nc.sync.dma_start(
                    gtile[:].rearrange("p a b -> p (a b)"), chunk_view(gum, base))

                wrows = bigp.tile([128, N_SUB, D], F32, tag='wrows')
                crows = bigp.tile([128, N_SUB, D], F32, tag='crows')
                for j in range(N_SUB):
                    nc.gpsimd.indirect_dma_start(
                        out=wrows[:, j, :], out_offset=None, in_=node[:],
                        in_offset=bass.IndirectOffsetOnAxis(
                            ap=idxw[:, j:j + 1], axis=0))
                    nc.gpsimd.indirect_dma_start(
                        out=crows[:, j, :], out_offset=None, in_=node[:],
                        in_offset=bass.IndirectOffsetOnAxis(
                            ap=idxc[:, j:j + 1], axis=0))
                xprod = bigp.tile([128, N_SUB, D], F32, tag='xprod')
                nc.vector.tensor_mul(
                    xprod[:].rearrange("p a b -> p (a b)"),
                    wrows[:].rearrange("p a b -> p (a b)"),
                    crows[:].rearrange("p a b -> p (a b)"))

                # matmuls: q = x @ W^T, pl = w_e @ W^T, PSUM-packed 8 subtiles/bank
                q_ps0 = psq.tile([128, 8 * K], F32, tag='q0')
                q_ps1 = psq.tile([128, 8 * K], F32, tag='q1')
                p_ps0 = psp.tile([128, 8 * K], F32, tag='p0')
                p_ps1 = psp.tile([128, 8 * K], F32, tag='p1')
                q_ps = [q_ps0, q_ps1]
                p_ps = [p_ps0, p_ps1]
                for j in range(N_SUB):
                    h, jj = j // 8, j % 8
                    xT_ps = pst.tile([128, 128], F32, tag='xT')
                    nc.tensor.transpose(out=xT_ps[:], in_=xprod[:, j, :],
                                        identity=ident[:])
                    xT = tpp.tile([128, 128], F32, tag='xTs')
                    nc.scalar.copy(xT[:], xT_ps[:])
                    nc.tensor.matmul(q_ps[h][:, jj * K:(jj + 1) * K],
                                     lhsT=xT[:], rhs=wT[:], start=True, stop=True)
                    wTr_ps = pst.tile([128, 128], F32, tag='wTr')
                    nc.tensor.transpose(out=wTr_ps[:], in_=wrows[:, j, :],
                                        identity=ident[:])
                    wTr = tpp.tile([128, 128], F32, tag='wTrs')
                    nc.scalar.copy(wTr[:], wTr_ps[:])
                    nc.tensor.matmul(p_ps[h][:, jj * K:(jj + 1) * K],
                                     lhsT=wTr[:], rhs=wT[:], start=True, stop=True)

                # softmax(q) -> y ; softmax(pl) -> prior ; argmax(q+g) -> kstar
                eq = midp.tile([128, N_SUB, K], F32, tag='eq')
                ep = midp.tile([128, N_SUB, K], F32, tag='ep')
                tq = midp.tile([128, N_SUB, K], F32, tag='tq')
                for h in range(2):
                    sl = slice(h * 8, (h + 1) * 8)
                    nc.scalar.activation(
                        eq[:, sl, :].rearrange("p a b -> p (a b)"), q_ps[h][:],
                        mybir.ActivationFunctionType.Exp)
                    nc.scalar.activation(
                        ep[:, sl, :].rearrange("p a b -> p (a b)"), p_ps[h][:],
                        mybir.ActivationFunctionType.Exp)
                    nc.vector.tensor_add(
                        tq[:, sl, :].rearrange("p a b -> p (a b)"), q_ps[h][:],
                        gtile[:, sl, :].rearrange("p a b -> p (a b)"))

                sq = smp.tile([128, N_SUB], F32, tag='sq')
                nc.vector.tensor_reduce(sq[:], eq[:], axis=mybir.AxisListType.X,
                                        op=mybir.AluOpType.add)
                rq = smp.tile([128, N_SUB], F32, tag='rq')
                nc.vector.reciprocal(rq[:], sq[:])
                ytile = midp.tile([128, N_SUB, K], F32, tag='yt')
                nc.vector.tensor_mul(ytile[:], eq[:],
                                     rq[:].rearrange("p a -> p a ()").to_broadcast([128, N_SUB, K]))
                nc.sync.dma_start(chunk_view(y_out, base),
                                  ytile[:].rearrange("p a b -> p (a b)"))

                sp = smp.tile([128, N_SUB], F32, tag='sp')
                nc.vector.tensor_reduce(sp[:], ep[:], axis=mybir.AxisListType.X,
                                        op=mybir.AluOpType.add)
                rp = smp.tile([128, N_SUB], F32, tag='rp')
                nc.vector.reciprocal(rp[:], sp[:])
                ptile = midp.tile([128, N_SUB, K], F32, tag='pt')
                nc.vector.tensor_mul(ptile[:], ep[:],
                                     rp[:].rearrange("p a -> p a ()").to_broadcast([128, N_SUB, K]))
                nc.sync.dma_start(chunk_view(p_out, base),
                                  ptile[:].rearrange("p a b -> p (a b)"))

                # argmax via grouped max + first-match reverse-iota trick
                m1 = smp.tile([128, N_SUB], F32, tag='m1')
                nc.vector.tensor_reduce(m1[:], tq[:], axis=mybir.AxisListType.X,
                                        op=mybir.AluOpType.max)
                eqm = midp.tile([128, N_SUB, K], F32, tag='eqm')
                nc.vector.tensor_tensor(
                    out=eqm[:], in0=tq[:],
                    in1=m1[:].rearrange("p a -> p a ()").to_broadcast([128, N_SUB, K]),
                    op=mybir.AluOpType.is_equal)
                sel = midp.tile([128, N_SUB, K], F32, tag='sel')
                nc.vector.tensor_mul(sel[:], eqm[:], iota_rev[:])
                m2 = smp.tile([128, N_SUB], F32, tag='m2')
                nc.vector.tensor_reduce(m2[:], sel[:], axis=mybir.AxisListType.X,
                                        op=mybir.AluOpType.max)
                kstar = smp.tile([128, N_SUB], F32, tag='kstar')
                nc.vector.tensor_scalar(out=kstar[:], in0=m2[:],
                                        scalar1=-1.0, scalar2=float(K),
                                        op0=mybir.AluOpType.mult,
                                        op1=mybir.AluOpType.add)

                # offsets: pos = c*64 + k*, neg = neg*64 + k*
                cf = smp.tile([128, N_SUB], F32, tag='cf')
                nc.vector.tensor_copy(cf[:], idxc[:])
                of_pos = smp.tile([128, N_SUB], F32, tag='ofp')
                nc.vector.tensor_scalar(out=of_pos[:], in0=cf[:],
                                        scalar1=float(K), scalar2=None,
                                        op0=mybir.AluOpType.mult)
                nc.vector.tensor_add(of_pos[:], of_pos[:], kstar[:])
                oi_pos = smp.tile([128, N_SUB], I32, tag='oip')
                nc.vector.tensor_copy(oi_pos[:], of_pos[:])

                nf = smp.tile([128, N_SUB, N_NEG], F32, tag='nf')
                nc.vector.tensor_copy(
                    nf[:].rearrange("p a b -> p (a b)"),
                    idxn[:].rearrange("p a b -> p (a b)"))
                of_neg = smp.tile([128, N_SUB, N_NEG], F32, tag='ofn')
                nc.vector.tensor_scalar(
                    out=of_neg[:].rearrange("p a b -> p (a b)"),
                    in0=nf[:].rearrange("p a b -> p (a b)"),
                    scalar1=float(K), scalar2=None, op0=mybir.AluOpType.mult)
                nc.vector.tensor_tensor(
                    out=of_neg[:], in0=of_neg[:],
                    in1=kstar[:].rearrange("p a -> p a ()").to_broadcast([128, N_SUB, N_NEG]),
                    op=mybir.AluOpType.add)
                oi_neg = smp.tile([128, N_SUB, N_NEG], I32, tag='oin')
                nc.vector.tensor_copy(
                    oi_neg[:].rearrange("p a b -> p (a b)"),
                    of_neg[:].rearrange("p a b -> p (a b)"))

                # phase B: element gathers of softplus tables
                scores = midp.tile([128, N_SUB, N_NEG + 1], F32, tag='sc')
                for j in range(N_SUB):
                    nc.gpsimd.indirect_dma_start(
                        out=scores[:, j, 0:1], out_offset=None, in_=gp_flat[:],
                        in_offset=bass.IndirectOffsetOnAxis(
                            ap=oi_pos[:, j:j + 1], axis=0))
                    for n in range(N_NEG):
                        nc.gpsimd.indirect_dma_start(
                            out=scores[:, j, n + 1:n + 2], out_offset=None,
                            in_=gn_flat[:],
                            in_offset=bass.IndirectOffsetOnAxis(
                                ap=oi_neg[:, j, n:n + 1], axis=0))
                csum = smp.tile([128, 1], F32, tag='csum')
                nc.vector.tensor_reduce(
                    csum[:], scores[:].rearrange("p a b -> p (a b)"),
                    axis=mybir.AxisListType.X, op=mybir.AluOpType.add)
                nc.vector.tensor_add(acc[:], acc[:], csum[:])

            nc.sync.dma_start(loss_out[:], acc[:])
    nc.finalize()
    return nc


# --------------------------------------------------------------------------- #
# host wrapper
# --------------------------------------------------------------------------- #

def kernel(w, c, neg, temp, gumbel_noise, node_emb, ctx_emb, community_w):
    w = np.ascontiguousarray(np.asarray(w, dtype=np.int64).astype(np.int32))
    c = np.ascontiguousarray(np.asarray(c, dtype=np.int64).astype(np.int32))
    neg = np.ascontiguousarray(np.asarray(neg, dtype=np.int64).astype(np.int32))
    gumbel = np.ascontiguousarray(np.asarray(gumbel_noise, dtype=np.float32))
    node = np.ascontiguousarray(np.asarray(node_emb, dtype=np.float32))
    ctx = np.ascontiguousarray(np.asarray(ctx_emb, dtype=np.float32))
    cw = np.ascontiguousarray(np.asarray(community_w, dtype=np.float32))
    tval = float(np.asarray(temp))
    assert tval > 0, "temp must be > 0 (argmax invariance)"

    rows = SIZE // N_CORES
    if 'l1' not in _COMPILED:
        _COMPILED['l1'] = build_l1(rows)
    res1 = _run_spmd(
        _COMPILED['l1'],
        [{'ctx': ctx[i * rows:(i + 1) * rows], 'cw': cw} for i in range(N_CORES)])
    gp = np.concatenate([r['gp'] for r in res1.results], axis=0)
    gn = np.concatenate([r['gn'] for r in res1.results], axis=0)

    # host relayout of index arrays: batch = core*B_CORE + ch*CHUNK + j*128 + p
    def relay(a):
        # a: [B] or [B, n] -> per core [N_CHUNK, 128, N_SUB(, n)]
        a2 = a.reshape(N_CORES, N_CHUNK, N_SUB, 128, *a.shape[1:])
        return np.ascontiguousarray(np.moveaxis(a2, 3, 2))

    wl, cl, negl = relay(w), relay(c), relay(neg)

    if 'l2' not in _COMPILED:
        _COMPILED['l2'] = build_l2()
    in_maps = []
    for i in range(N_CORES):
        in_maps.append({
            'node': node, 'gp': gp, 'gn': gn, 'cw': cw,
            'wl': wl[i], 'cl': cl[i], 'negl': negl[i],
            'gum': gumbel[i * B_CORE:(i + 1) * B_CORE],
        })
    res2 = _run_spmd(_COMPILED['l2'], in_maps)

    y = np.concatenate([r['y_out'] for r in res2.results], axis=0)
    prior = np.concatenate([r['p_out'] for r in res2.results], axis=0)
    loss = np.float32(sum(float(r['loss_out'].sum()) for r in res2.results) / B)
    return loss, y, prior


# revision 7
# speedup vs baseline: 1.2514x; 1.2514x over previous
"""Trainium2 Bass kernel for nn_GCNModelGumbel (gumbel-softmax skip-gram loss).

Math (matching reference.py, with z = stop_gradient(y_hard - y_soft) + y_soft
== y_hard numerically == onehot(argmax(q + gumbel)) for temp > 0):

  q[b]      = (node_emb[w_b] * node_emb[c_b]) @ W^T          [B, 64]
  y         = softmax(q)                                      (output 2)
  prior     = softmax(node_emb[w_b] @ W^T)                    (output 3)
  k*_b      = argmax(q + gumbel)
  loss      = mean_b[ sp(-proj[c_b, k*]) + 0.2 * sum_n sp(proj[neg_bn, k*]) ]
  where proj = ctx_emb @ W^T and sp = softplus                (output 1)

Two SPMD launches on 8 cores:
  L1: each core computes a 1/8 slice of Gp = sp(-proj), Gn2 = 0.2*sp(proj).
  L2: data-parallel over batch; per 2048-batch chunk: indirect row-gathers of
      node_emb for w/c, on-chip matmuls + softmaxes + argmax, then indirect
      element gathers of Gp/Gn2 at flat offsets idx*64 + k*.
"""
import sys
sys.path.insert(0, '/opt/trn_rl_repo')

import numpy as np

import concourse.bacc as bacc
import concourse.bass as bass
import concourse.mybir as mybir
import concourse.tile as tile
from concourse.masks import make_identity

SIZE = 100000
D = 128
K = 64
B = 131072
N_NEG = 5
N_CORES = 8
B_CORE = B // N_CORES            # 16384
CHUNK = 2048                     # batches per chunk
N_CHUNK = B_CORE // CHUNK        # 8
N_SUB = CHUNK // 128             # 16 subtiles per chunk

F32 = mybir.dt.float32
I32 = mybir.dt.int32

_COMPILED = {}


def _run_spmd(nc, in_maps):
    from concourse.bass_utils import run_bass_kernel_spmd
    return run_bass_kernel_spmd(nc, in_maps, core_ids=list(range(N_CORES)))


# --------------------------------------------------------------------------- #
# Launch 1: per-core slice of Gp / Gn2 tables
# --------------------------------------------------------------------------- #

def build_l1(rows):
    """rows = number of ctx_emb rows this core handles (SIZE/8 = 12500)."""
    nc = bacc.Bacc(None, target_bir_lowering=False)
    ctx = nc.dram_tensor('ctx', [rows, D], F32, kind='ExternalInput')
    cw = nc.dram_tensor('cw', [K, D], F32, kind='ExternalInput')
    gp = nc.dram_tensor('gp', [rows, K], F32, kind='ExternalOutput')
    gn = nc.dram_tensor('gn', [rows, K], F32, kind='ExternalOutput')

    GRP = 4  # 128-row tiles per group; ACT ops amortized across the group
    with tile.TileContext(nc) as tc:
        with tc.tile_pool(name='const', bufs=1) as cpool, \
             tc.tile_pool(name='work', bufs=3) as pool, \
             tc.tile_pool(name='tps', bufs=4) as tpool, \
             tc.tile_pool(name='ps', bufs=3, space='PSUM') as psum, \
             tc.tile_pool(name='ps2', bufs=2, space='PSUM') as psum2:
            ident = cpool.tile([128, 128], F32)
            make_identity(nc, ident[:])
            cw_t = cpool.tile([K, D], F32)
            nc.sync.dma_start(cw_t[:], cw[:])
            wT_ps = psum.tile([128, 128], F32, tag='xT')
            nc.tensor.transpose(out=wT_ps[:, :K], in_=cw_t[:], identity=ident[:K, :K])
            wT = cpool.tile([128, K], F32)
            nc.vector.tensor_copy(wT[:], wT_ps[:, :K])

            r0 = 0
            while r0 < rows:
                gsz = min(GRP * 128, rows - r0)
                nt = (gsz + 127) // 128
                rowst = pool.tile([128, GRP, D], F32, tag='rows')
                full = (gsz == GRP * 128)
                if full:
                    nc.sync.dma_start(
                        rowst[:].rearrange("p a b -> p (a b)"),
                        bass.AP(ctx, r0 * D, [[D, 128], [128 * D, GRP], [1, D]]))
                pr_ps = psum2.tile([128, GRP * K], F32, tag='proj')
                for t in range(nt):
                    p = min(128, gsz - t * 128)
                    if not full:
                        rt = tpool.tile([128, D], F32, tag='rrow')
                        nc.sync.dma_start(rt[:p, :], ctx[r0 + t * 128:r0 + t * 128 + p, :])
                        src_ap = rt[:p, :]
                    else:
                        src_ap = rowst[:, t, :]
                    xT_ps = psum.tile([128, 128], F32, tag='xT')
                    nc.tensor.transpose(out=xT_ps[:, :p], in_=src_ap,
                                        identity=ident[:p, :p])
                    xT = tpool.tile([128, 128], F32, tag='xTs')
                    nc.vector.tensor_copy(xT[:, :p], xT_ps[:, :p])
                    nc.tensor.matmul(pr_ps[:p, t * K:(t + 1) * K],
                                     lhsT=xT[:, :p], rhs=wT[:],
                                     start=True, stop=True)
                # softplus both signs over the whole group [128, nt*K]
                w_ = nt * K
                e1 = pool.tile([128, GRP * K], F32, tag='e1')
                nc.scalar.activation(e1[:, :w_], pr_ps[:, :w_],
                                     mybir.ActivationFunctionType.Exp, scale=-1.0)
                g1 = pool.tile([128, GRP * K], F32, tag='g1')
                nc.scalar.activation(g1[:, :w_], e1[:, :w_],
                                     mybir.ActivationFunctionType.Ln, bias=1.0)
                e2 = pool.tile([128, GRP * K], F32, tag='e2')
                nc.scalar.activation(e2[:, :w_], pr_ps[:, :w_],
                                     mybir.ActivationFunctionType.Exp)
                g2 = pool.tile([128, GRP * K], F32, tag='g2')
                nc.scalar.activation(g2[:, :w_], e2[:, :w_],
                                     mybir.ActivationFunctionType.Ln, bias=1.0)
                g2s = pool.tile([128, GRP * K], F32, tag='g2s')
                nc.vector.tensor_scalar_mul(g2s[:, :w_], g2[:, :w_], 1.0 / N_NEG)
                for t in range(nt):
                    p = min(128, gsz - t * 128)
                    nc.sync.dma_start(gp[r0 + t * 128:r0 + t * 128 + p, :],
                                      g1[:p, t * K:(t + 1) * K])
                    nc.sync.dma_start(gn[r0 + t * 128:r0 + t * 128 + p, :],
                                      g2s[:p, t * K:(t + 1) * K])
                r0 += gsz
    nc.finalize()
    return nc


# --------------------------------------------------------------------------- #
# Launch 2: main kernel (per-core batch shard)
# --------------------------------------------------------------------------- #

def build_l2():
    nc = bacc.Bacc(None, target_bir_lowering=False)
    node = nc.dram_tensor('node', [SIZE, D], F32, kind='ExternalInput')
    gp = nc.dram_tensor('gp', [SIZE, K], F32, kind='ExternalInput')
    gn = nc.dram_tensor('gn', [SIZE, K], F32, kind='ExternalInput')
    cw = nc.dram_tensor('cw', [K, D], F32, kind='ExternalInput')
    # host-relaid index tensors: [nchunk, 128, ...] with batch = base + j*128 + p
    wl = nc.dram_tensor('wl', [N_CHUNK, 128, N_SUB], I32, kind='ExternalInput')
    cl = nc.dram_tensor('cl', [N_CHUNK, 128, N_SUB], I32, kind='ExternalInput')
    negl = nc.dram_tensor('negl', [N_CHUNK, 128, N_SUB, N_NEG], I32,
                          kind='ExternalInput')
    gum = nc.dram_tensor('gum', [B_CORE, K], F32, kind='ExternalInput')
    y_out = nc.dram_tensor('y_out', [B_CORE, K], F32, kind='ExternalOutput')
    p_out = nc.dram_tensor('p_out', [B_CORE, K], F32, kind='ExternalOutput')
    loss_out = nc.dram_tensor('loss_out', [128, 1], F32, kind='ExternalOutput')


    def chunk_view(t, base):
        # [B_CORE, K] dram tensor viewed as [p=128, a=N_SUB, b=K] for batch
        # row = base + a*128 + p
        return bass.AP(t, base * K, [[K, 128], [128 * K, N_SUB], [1, K]])
    gp_flat = gp.rearrange("v k -> (v k) ()")
    gn_flat = gn.rearrange("v k -> (v k) ()")

    with tile.TileContext(nc) as tc:
        with tc.tile_pool(name='const', bufs=1) as cpool, \
             tc.tile_pool(name='io', bufs=2) as iop, \
             tc.tile_pool(name='big', bufs=2) as bigp, \
             tc.tile_pool(name='mid', bufs=2) as midp, \
             tc.tile_pool(name='sm', bufs=3) as smp, \
             tc.tile_pool(name='tp', bufs=4) as tpp, \
             tc.tile_pool(name='psq', bufs=1, space='PSUM') as psq, \
             tc.tile_pool(name='psp', bufs=1, space='PSUM') as psp, \
             tc.tile_pool(name='pst', bufs=2, space='PSUM') as pst:
            ident = cpool.tile([128, 128], F32)
            make_identity(nc, ident[:])
            cw_t = cpool.tile([K, D], F32)
            nc.sync.dma_start(cw_t[:], cw[:])
            wT_ps = pst.tile([128, K], F32, tag='xT')
            nc.tensor.transpose(out=wT_ps[:], in_=cw_t[:], identity=ident[:K, :K])
            wT = cpool.tile([128, K], F32)
            nc.vector.tensor_copy(wT[:], wT_ps[:])
            # iota along K repeated per subtile: [128, N_SUB, K], value = k
            iota_i = cpool.tile([128, N_SUB * K], I32)
            nc.gpsimd.iota(iota_i[:], pattern=[[0, N_SUB], [1, K]],
                           channel_multiplier=0)
            iota_f = cpool.tile([128, N_SUB, K], F32)
            nc.vector.tensor_copy(iota_f[:].rearrange("p a b -> p (a b)"), iota_i[:])
            # reversed iota 64-k (for first-argmax tie-break): value = K - k
            iota_rev = cpool.tile([128, N_SUB, K], F32)
            nc.vector.tensor_scalar(
                out=iota_rev[:].rearrange("p a b -> p (a b)"),
                in0=iota_f[:].rearrange("p a b -> p (a b)"),
                scalar1=-1.0, scalar2=float(K),
                op0=mybir.AluOpType.mult, op1=mybir.AluOpType.add)
            acc = cpool.tile([128, 1], F32)
            nc.vector.memset(acc[:], 0.0)

            for ch in range(N_CHUNK):
                base = ch * CHUNK
                idxw = iop.tile([128, N_SUB], I32, tag='idxw')
                nc.sync.dma_start(idxw[:], wl[ch])
                idxc = iop.tile([128, N_SUB], I32, tag='idxc')
                nc.sync.dma_start(idxc[:], cl[ch])
                idxn = iop.tile([128, N_SUB, N_NEG], I32, tag='idxn')
                nc.sync.dma_start(idxn[:], negl[ch])
                gtile = bigp.tile([128, N_SUB, K], F32, tag='gum')
                nc.sync.dma_start(
                    gtile[:].rearrange("p a b -> p (a b)"), chunk_view(gum, base))

                wrows = bigp.tile([128, N_SUB, D], F32, tag='wrows')
                crows = bigp.tile([128, N_SUB, D], F32, tag='crows')
                for j in range(N_SUB):
                    nc.gpsimd.indirect_dma_start(
                        out=wrows[:, j, :], out_offset=None, in_=node[:],
                        in_offset=bass.IndirectOffsetOnAxis(
                            ap=idxw[:, j:j + 1], axis=0))
                    nc.gpsimd.indirect_dma_start(
                        out=crows[:, j, :], out_offset=None, in_=node[:],
                        in_offset=bass.IndirectOffsetOnAxis(
                            ap=idxc[:, j:j + 1], axis=0))
                xprod = bigp.tile([128, N_SUB, D], F32, tag='xprod')
                nc.vector.tensor_mul(
                    xprod[:].rearrange("p a b -> p (a b)"),
                    wrows[:].rearrange("p a b -> p (a b)"),
                    crows[:].rearrange("p a b -> p (a b)"))

                # matmuls: q = x @ W^T, pl = w_e @ W^T, PSUM-packed 8 subtiles/bank
                q_ps0 = psq.tile([128, 8 * K], F32, tag='q0')
                q_ps1 = psq.tile([128, 8 * K], F32, tag='q1')
                p_ps0 = psp.tile([128, 8 * K], F32, tag='p0')
                p_ps1 = psp.tile([128, 8 * K], F32, tag='p1')
                q_ps = [q_ps0, q_ps1]
                p_ps = [p_ps0, p_ps1]
                for j in range(N_SUB):
                    h, jj = j // 8, j % 8
                    xT_ps = pst.tile([128, 128], F32, tag='xT')
                    nc.tensor.transpose(out=xT_ps[:], in_=xprod[:, j, :],
                                        identity=ident[:])
                    xT = tpp.tile([128, 128], F32, tag='xTs')
                    nc.scalar.copy(xT[:], xT_ps[:])
                    nc.tensor.matmul(q_ps[h][:, jj * K:(jj + 1) * K],
                                     lhsT=xT[:], rhs=wT[:], start=True, stop=True)
                    wTr_ps = pst.tile([128, 128], F32, tag='wTr')
                    nc.tensor.transpose(out=wTr_ps[:], in_=wrows[:, j, :],
                                        identity=ident[:])
                    wTr = tpp.tile([128, 128], F32, tag='wTrs')
                    nc.scalar.copy(wTr[:], wTr_ps[:])
                    nc.tensor.matmul(p_ps[h][:, jj * K:(jj + 1) * K],
                                     lhsT=wTr[:], rhs=wT[:], start=True, stop=True)

                # softmax(q) -> y ; softmax(pl) -> prior ; argmax(q+g) -> kstar
                eq = midp.tile([128, N_SUB, K], F32, tag='eq')
                ep = midp.tile([128, N_SUB, K], F32, tag='ep')
                tq = midp.tile([128, N_SUB, K], F32, tag='tq')
                for h in range(2):
                    sl = slice(h * 8, (h + 1) * 8)
                    nc.scalar.activation(
                        eq[:, sl, :].rearrange("p a b -> p (a b)"), q_ps[h][:],
                        mybir.ActivationFunctionType.Exp)
                    nc.scalar.activation(
                        ep[:, sl, :].rearrange("p a b -> p (a b)"), p_ps[h][:],
                        mybir.ActivationFunctionType.Exp)
                    nc.vector.tensor_add(
                        tq[:, sl, :].rearrange("p a b -> p (a b)"), q_ps[h][:],
                        gtile[:, sl, :].rearrange("p a b -> p (a b)"))

                sq = smp.tile([128, N_SUB], F32, tag='sq')
                nc.vector.tensor_reduce(sq[:], eq[:], axis=mybir.AxisListType.X,
                                        op=mybir.AluOpType.add)
                rq = smp.tile([128, N_SUB], F32, tag='rq')
                nc.vector.reciprocal(rq[:], sq[:])
                ytile = midp.tile([128, N_SUB, K], F32, tag='yt')
                nc.vector.tensor_mul(ytile[:], eq[:],
                                     rq[:].rearrange("p a -> p a ()").to_broadcast([128, N_SUB, K]))
                nc.sync.dma_start(chunk_view(y_out, base),
                                  ytile[:].rearrange("p a b -> p (a b)"))

                sp = smp.tile([128, N_SUB], F32, tag='sp')
                nc.vector.tensor_reduce(sp[:], ep[:], axis=mybir.AxisListType.X,
                                        op=mybir.AluOpType.add)
                rp = smp.tile([128, N_SUB], F32, tag='rp')
                nc.vector.reciprocal(rp[:], sp[:])
                ptile = midp.tile([128, N_SUB, K], F32, tag='pt')
                nc.vector.tensor_mul(ptile[:], ep[:],
                                     rp[:].rearrange("p a -> p a ()").to_broadcast([128, N_SUB, K]))
                nc.sync.dma_start(chunk_view(p_out, base),
                                  ptile[:].rearrange("p a b -> p (a b)"))

                # argmax via grouped max + first-match reverse-iota trick
                m1 = smp.tile([128, N_SUB], F32, tag='m1')
                nc.vector.tensor_reduce(m1[:], tq[:], axis=mybir.AxisListType.X,
                                        op=mybir.AluOpType.max)
                eqm = midp.tile([128, N_SUB, K], F32, tag='eqm')
                nc.vector.tensor_tensor(
                    out=eqm[:], in0=tq[:],
                    in1=m1[:].rearrange("p a -> p a ()").to_broadcast([128, N_SUB, K]),
                    op=mybir.AluOpType.is_equal)
                sel = midp.tile([128, N_SUB, K], F32, tag='sel')
                nc.vector.tensor_mul(sel[:], eqm[:], iota_rev[:])
                m2 = smp.tile([128, N_SUB], F32, tag='m2')
                nc.vector.tensor_reduce(m2[:], sel[:], axis=mybir.AxisListType.X,
                                        op=mybir.AluOpType.max)
                kstar = smp.tile([128, N_SUB], F32, tag='kstar')
                nc.vector.tensor_scalar(out=kstar[:], in0=m2[:],
                                        scalar1=-1.0, scalar2=float(K),
                                        op0=mybir.AluOpType.mult,
                                        op1=mybir.AluOpType.add)

                # offsets: pos = c*64 + k*, neg = neg*64 + k*
                cf = smp.tile([128, N_SUB], F32, tag='cf')
                nc.vector.tensor_copy(cf[:], idxc[:])
                of_pos = smp.tile([128, N_SUB], F32, tag='ofp')
                nc.vector.tensor_scalar(out=of_pos[:], in0=cf[:],
                                        scalar1=float(K), scalar2=None,
                                        op0=mybir.AluOpType.mult)
                nc.vector.tensor_add(of_pos[:], of_pos[:], kstar[:])
                oi_pos = smp.tile([128, N_SUB], I32, tag='oip')
                nc.vector.tensor_copy(oi_pos[:], of_pos[:])

                nf = smp.tile([128, N_SUB, N_NEG], F32, tag='nf')
                nc.vector.tensor_copy(
                    nf[:].rearrange("p a b -> p (a b)"),
                    idxn[:].rearrange("p a b -> p (a b)"))
                of_neg = smp.tile([128, N_SUB, N_NEG], F32, tag='ofn')
                nc.vector.tensor_scalar(
                    out=of_neg[:].rearrange("p a b -> p (a b)"),
                    in0=nf[:].rearrange("p a b -> p (a b)"),
                    scalar1=float(K), scalar2=None, op0=mybir.AluOpType.mult)
                nc.vector.tensor_tensor(
                    out=of_neg[:], in0=of_neg[:],
                    in1=kstar[:].rearrange("p a -> p a ()").to_broadcast([128, N_SUB, N_NEG]),
                    op=mybir.AluOpType.add)
                oi_neg = smp.tile([128, N_SUB, N_NEG], I32, tag='oin')
                nc.vector.tensor_copy(
                    oi_neg[:].rearrange("p a b -> p (a b)"),
                    of_neg[:].rearrange("p a b -> p (a b)"))

                # phase B: element gathers of softplus tables
                scores = midp.tile([128, N_SUB, N_NEG + 1], F32, tag='sc')
                for j in range(N_SUB):
                    nc.gpsimd.indirect_dma_start(
                        out=scores[:, j, 0:1], out_offset=None, in_=gp_flat[:],
                        in_offset=bass.IndirectOffsetOnAxis(
                            ap=oi_pos[:, j:j + 1], axis=0))
                    for n in range(N_NEG):
                        nc.gpsimd.indirect_dma_start(
                            out=scores[:, j, n + 1:n + 2], out_offset=None,
                            in_=gn_flat[:],
                            in_offset=bass.IndirectOffsetOnAxis(
                                ap=oi_neg[:, j, n:n + 1], axis=0))
                csum = smp.tile([128, 1], F32, tag='csum')
                nc.vector.tensor_reduce(
                    csum[:], scores[:].rearrange("p a b -> p (a b)"),
                    axis=mybir.AxisListType.X, op=mybir.AluOpType.add)
                nc.vector.tensor_add(acc[:], acc[:], csum[:])

            nc.sync.dma_start(loss_out[:], acc[:])
    nc.finalize()
    return nc


# --------------------------------------------------------------------------- #
# host wrapper
# --------------------------------------------------------------------------- #

def kernel(w, c, neg, temp, gumbel_noise, node_emb, ctx_emb, community_w):
    w = np.ascontiguousarray(np.asarray(w, dtype=np.int64).astype(np.int32))
    c = np.ascontiguousarray(np.asarray(c, dtype=np.int64).astype(np.int32))
    neg = np.ascontiguousarray(np.asarray(neg, dtype=np.int64).astype(np.int32))
    gumbel = np.ascontiguousarray(np.asarray(gumbel_noise, dtype=np.float32))
    node = np.ascontiguousarray(np.asarray(node_emb, dtype=np.float32))
    ctx = np.ascontiguousarray(np.asarray(ctx_emb, dtype=np.float32))
    cw = np.ascontiguousarray(np.asarray(community_w, dtype=np.float32))
    tval = float(np.asarray(temp))
    assert tval > 0, "temp must be > 0 (argmax invariance)"

    rows = SIZE // N_CORES
    if 'l1' not in _COMPILED:
        _COMPILED['l1'] = build_l1(rows)
    res1 = _run_spmd(
        _COMPILED['l1'],
        [{'ctx': ctx[i * rows:(i + 1) * rows], 'cw': cw} for i in range(N_CORES)])
    gp = np.concatenate([r['gp'] for r in res1.results], axis=0)
    gn = np.concatenate([r['gn'] for r in res1.results], axis=0)

    # host relayout of index arrays: batch = core*B_CORE + ch*CHUNK + j*128 + p
    def relay(a):
        # a: [B] or [B, n] -> per core [N_CHUNK, 128, N_SUB(, n)]
        a2 = a.reshape(N_CORES, N_CHUNK, N_SUB, 128, *a.shape[1:])
        return np.ascontiguousarray(np.moveaxis(a2, 3, 2))

    wl, cl, negl = relay(w), relay(c), relay(neg)

    if 'l2' not in _COMPILED:
        _COMPILED['l2'] = build_l2()
    in_maps = []
    for i in range(N_CORES):
        in_maps.append({
            'node': node, 'gp': gp, 'gn': gn, 'cw': cw,
            'wl': wl[i], 'cl': cl[i], 'negl': negl[i],
            'gum': gumbel[i * B_CORE:(i + 1) * B_CORE],
        })
    res2 = _run_spmd(_COMPILED['l2'], in_maps)

    y = np.concatenate([r['y_out'] for r in res2.results], axis=0)
    prior = np.concatenate([r['p_out'] for r in res2.results], axis=0)
    loss = np.float32(sum(float(r['loss_out'].sum()) for r in res2.results) / B)
    return loss, y, prior


# revision 8
# speedup vs baseline: 1.2665x; 1.0120x over previous
"""Trainium2 Bass kernel for nn_GCNModelGumbel (gumbel-softmax skip-gram loss).

Math (matching reference.py, with z = stop_gradient(y_hard - y_soft) + y_soft
== y_hard numerically == onehot(argmax(q + gumbel)) for temp > 0):

  q[b]      = (node_emb[w_b] * node_emb[c_b]) @ W^T          [B, 64]
  y         = softmax(q)                                      (output 2)
  prior     = softmax(node_emb[w_b] @ W^T)                    (output 3)
  k*_b      = argmax(q + gumbel)
  loss      = mean_b[ sp(-proj[c_b, k*]) + 0.2 * sum_n sp(proj[neg_bn, k*]) ]
  where proj = ctx_emb @ W^T and sp = softplus                (output 1)

Two SPMD launches on 8 cores:
  L1: each core computes a 1/8 slice of Gp = sp(-proj), Gn2 = 0.2*sp(proj).
  L2: data-parallel over batch; per 2048-batch chunk: indirect row-gathers of
      node_emb for w/c, on-chip matmuls + softmaxes + argmax, then indirect
      element gathers of Gp/Gn2 at flat offsets idx*64 + k*.
"""
import sys
sys.path.insert(0, '/opt/trn_rl_repo')

import numpy as np

import concourse.bacc as bacc
import concourse.bass as bass
import concourse.mybir as mybir
import concourse.tile as tile
from concourse.masks import make_identity

SIZE = 100000
D = 128
K = 64
B = 131072
N_NEG = 5
N_CORES = 8
B_CORE = B // N_CORES            # 16384
CHUNK = 2048                     # batches per chunk
N_CHUNK = B_CORE // CHUNK        # 8
N_SUB = CHUNK // 128             # 16 subtiles per chunk

F32 = mybir.dt.float32
I32 = mybir.dt.int32

_COMPILED = {}


def _run_spmd(nc, in_maps):
    from concourse.bass_utils import run_bass_kernel_spmd
    return run_bass_kernel_spmd(nc, in_maps, core_ids=list(range(N_CORES)))


# --------------------------------------------------------------------------- #
# Launch 1: per-core slice of Gp / Gn2 tables
# --------------------------------------------------------------------------- #

def build_l1(rows):
    """rows = number of ctx_emb rows this core handles (SIZE/8 = 12500)."""
    nc = bacc.Bacc(None, target_bir_lowering=False)
    ctx = nc.dram_tensor('ctx', [rows, D], F32, kind='ExternalInput')
    cw = nc.dram_tensor('cw', [K, D], F32, kind='ExternalInput')
    gp = nc.dram_tensor('gp', [rows, K], F32, kind='ExternalOutput')
    gn = nc.dram_tensor('gn', [rows, K], F32, kind='ExternalOutput')

    GRP = 4  # 128-row tiles per group; ACT ops amortized across the group
    with tile.TileContext(nc) as tc:
        with tc.tile_pool(name='const', bufs=1) as cpool, \
             tc.tile_pool(name='work', bufs=4) as pool, \
             tc.tile_pool(name='tps', bufs=6) as tpool, \
             tc.tile_pool(name='ps', bufs=4, space='PSUM') as psum, \
             tc.tile_pool(name='ps2', bufs=3, space='PSUM') as psum2:
            ident = cpool.tile([128, 128], F32)
            make_identity(nc, ident[:])
            cw_t = cpool.tile([K, D], F32)
            nc.sync.dma_start(cw_t[:], cw[:])
            wT_ps = psum.tile([128, 128], F32, tag='xT')
            nc.tensor.transpose(out=wT_ps[:, :K], in_=cw_t[:], identity=ident[:K, :K])
            wT = cpool.tile([128, K], F32)
            nc.vector.tensor_copy(wT[:], wT_ps[:, :K])

            r0 = 0
            while r0 < rows:
                gsz = min(GRP * 128, rows - r0)
                nt = (gsz + 127) // 128
                rowst = pool.tile([128, GRP, D], F32, tag='rows')
                full = (gsz == GRP * 128)
                if full:
                    nc.sync.dma_start(
                        rowst[:].rearrange("p a b -> p (a b)"),
                        bass.AP(ctx, r0 * D, [[D, 128], [128 * D, GRP], [1, D]]))
                pr_ps = psum2.tile([128, GRP * K], F32, tag='proj')
                for t in range(nt):
                    p = min(128, gsz - t * 128)
                    if not full:
                        rt = tpool.tile([128, D], F32, tag='rrow')
                        nc.sync.dma_start(rt[:p, :], ctx[r0 + t * 128:r0 + t * 128 + p, :])
                        src_ap = rt[:p, :]
                    else:
                        src_ap = rowst[:, t, :]
                    xT_ps = psum.tile([128, 128], F32, tag='xT')
                    nc.tensor.transpose(out=xT_ps[:, :p], in_=src_ap,
                                        identity=ident[:p, :p])
                    xT = tpool.tile([128, 128], F32, tag='xTs')
                    nc.vector.tensor_copy(xT[:, :p], xT_ps[:, :p])
                    nc.tensor.matmul(pr_ps[:p, t * K:(t + 1) * K],
                                     lhsT=xT[:, :p], rhs=wT[:],
                                     start=True, stop=True)
                # softplus both signs over the whole group [128, nt*K]
                w_ = nt * K
                e1 = pool.tile([128, GRP * K], F32, tag='e1')
                nc.scalar.activation(e1[:, :w_], pr_ps[:, :w_],
                                     mybir.ActivationFunctionType.Exp, scale=-1.0)
                g1 = pool.tile([128, GRP * K], F32, tag='g1')
                nc.scalar.activation(g1[:, :w_], e1[:, :w_],
                                     mybir.ActivationFunctionType.Ln, bias=1.0)
                e2 = pool.tile([128, GRP * K], F32, tag='e2')
                nc.scalar.activation(e2[:, :w_], pr_ps[:, :w_],
                                     mybir.ActivationFunctionType.Exp)
                g2 = pool.tile([128, GRP * K], F32, tag='g2')
                nc.scalar.activation(g2[:, :w_], e2[:, :w_],
                                     mybir.ActivationFunctionType.Ln, bias=1.0)
                g2s = pool.tile([128, GRP * K], F32, tag='g2s')
                nc.vector.tensor_scalar_mul(g2s[:, :w_], g2[:, :w_], 1.0 / N_NEG)
                for t in range(nt):
                    p = min(128, gsz - t * 128)
                    nc.sync.dma_start(gp[r0 + t * 128:r0 + t * 128 + p, :],
                                      g1[:p, t * K:(t + 1) * K])
                    nc.sync.dma_start(gn[r0 + t * 128:r0 + t * 128 + p, :],
                                      g2s[:p, t * K:(t + 1) * K])
                r0 += gsz
    nc.finalize()
    return nc


# --------------------------------------------------------------------------- #
# Launch 2: main kernel (per-core batch shard)
# --------------------------------------------------------------------------- #

def build_l2():
    nc = bacc.Bacc(None, target_bir_lowering=False)
    node = nc.dram_tensor('node', [SIZE, D], F32, kind='ExternalInput')
    gp = nc.dram_tensor('gp', [SIZE, K], F32, kind='ExternalInput')
    gn = nc.dram_tensor('gn', [SIZE, K], F32, kind='ExternalInput')
    cw = nc.dram_tensor('cw', [K, D], F32, kind='ExternalInput')
    # host-relaid index tensors: [nchunk, 128, ...] with batch = base + j*128 + p
    wl = nc.dram_tensor('wl', [N_CHUNK, 128, N_SUB], I32, kind='ExternalInput')
    cl = nc.dram_tensor('cl', [N_CHUNK, 128, N_SUB], I32, kind='ExternalInput')
    negl = nc.dram_tensor('negl', [N_CHUNK, 128, N_SUB, N_NEG], I32,
                          kind='ExternalInput')
    gum = nc.dram_tensor('gum', [B_CORE, K], F32, kind='ExternalInput')
    y_out = nc.dram_tensor('y_out', [B_CORE, K], F32, kind='ExternalOutput')
    p_out = nc.dram_tensor('p_out', [B_CORE, K], F32, kind='ExternalOutput')
    loss_out = nc.dram_tensor('loss_out', [128, 1], F32, kind='ExternalOutput')


    def chunk_view(t, base):
        # [B_CORE, K] dram tensor viewed as [p=128, a=N_SUB, b=K] for batch
        # row = base + a*128 + p
        return bass.AP(t, base * K, [[K, 128], [128 * K, N_SUB], [1, K]])
    gp_flat = gp.rearrange("v k -> (v k) ()")
    gn_flat = gn.rearrange("v k -> (v k) ()")

    with tile.TileContext(nc) as tc:
        with tc.tile_pool(name='const', bufs=1) as cpool, \
             tc.tile_pool(name='io', bufs=2) as iop, \
             tc.tile_pool(name='big', bufs=2) as bigp, \
             tc.tile_pool(name='mid', bufs=2) as midp, \
             tc.tile_pool(name='sm', bufs=3) as smp, \
             tc.tile_pool(name='tp', bufs=4) as tpp, \
             tc.tile_pool(name='psq', bufs=1, space='PSUM') as psq, \
             tc.tile_pool(name='psp', bufs=1, space='PSUM') as psp, \
             tc.tile_pool(name='pst', bufs=2, space='PSUM') as pst:
            ident = cpool.tile([128, 128], F32)
            make_identity(nc, ident[:])
            cw_t = cpool.tile([K, D], F32)
            nc.sync.dma_start(cw_t[:], cw[:])
            wT_ps = pst.tile([128, K], F32, tag='xT')
            nc.tensor.transpose(out=wT_ps[:], in_=cw_t[:], identity=ident[:K, :K])
            wT = cpool.tile([128, K], F32)
            nc.vector.tensor_copy(wT[:], wT_ps[:])
            # iota along K repeated per subtile: [128, N_SUB, K], value = k
            iota_i = cpool.tile([128, N_SUB * K], I32)
            nc.gpsimd.iota(iota_i[:], pattern=[[0, N_SUB], [1, K]],
                           channel_multiplier=0)
            iota_f = cpool.tile([128, N_SUB, K], F32)
            nc.vector.tensor_copy(iota_f[:].rearrange("p a b -> p (a b)"), iota_i[:])
            # reversed iota 64-k (for first-argmax tie-break): value = K - k
            iota_rev = cpool.tile([128, N_SUB, K], F32)
            nc.vector.tensor_scalar(
                out=iota_rev[:].rearrange("p a b -> p (a b)"),
                in0=iota_f[:].rearrange("p a b -> p (a b)"),
                scalar1=-1.0, scalar2=float(K),
                op0=mybir.AluOpType.mult, op1=mybir.AluOpType.add)
            acc = cpool.tile([128, 1], F32)
            nc.vector.memset(acc[:], 0.0)

            for ch in range(N_CHUNK):
                base = ch * CHUNK
                idxw = iop.tile([128, N_SUB], I32, tag='idxw')
                nc.sync.dma_start(idxw[:], wl[ch])
                idxc = iop.tile([128, N_SUB], I32, tag='idxc')
                nc.sync.dma_start(idxc[:], cl[ch])
                idxn = iop.tile([128, N_SUB, N_NEG], I32, tag='idxn')
                nc.sync.dma_start(idxn[:], negl[ch])
                gtile = bigp.tile([128, N_SUB, K], F32, tag='gum')
                nc.sync.dma_start(
                    gtile[:].rearrange("p a b -> p (a b)"), chunk_view(gum, base))

                wrows = bigp.tile([128, N_SUB, D], F32, tag='wrows')
                crows = bigp.tile([128, N_SUB, D], F32, tag='crows')
                for j in range(N_SUB):
                    nc.gpsimd.indirect_dma_start(
                        out=wrows[:, j, :], out_offset=None, in_=node[:],
                        in_offset=bass.IndirectOffsetOnAxis(
                            ap=idxw[:, j:j + 1], axis=0))
                    nc.gpsimd.indirect_dma_start(
                        out=crows[:, j, :], out_offset=None, in_=node[:],
                        in_offset=bass.IndirectOffsetOnAxis(
                            ap=idxc[:, j:j + 1], axis=0))
                xprod = bigp.tile([128, N_SUB, D], F32, tag='xprod')
                nc.vector.tensor_mul(
                    xprod[:].rearrange("p a b -> p (a b)"),
                    wrows[:].rearrange("p a b -> p (a b)"),
                    crows[:].rearrange("p a b -> p (a b)"))

                # matmuls: q = x @ W^T, pl = w_e @ W^T, PSUM-packed 8 subtiles/bank
                q_ps0 = psq.tile([128, 8 * K], F32, tag='q0')
                q_ps1 = psq.tile([128, 8 * K], F32, tag='q1')
                p_ps0 = psp.tile([128, 8 * K], F32, tag='p0')
                p_ps1 = psp.tile([128, 8 * K], F32, tag='p1')
                q_ps = [q_ps0, q_ps1]
                p_ps = [p_ps0, p_ps1]
                for j in range(N_SUB):
                    h, jj = j // 8, j % 8
                    xT_ps = pst.tile([128, 128], F32, tag='xT')
                    nc.tensor.transpose(out=xT_ps[:], in_=xprod[:, j, :],
                                        identity=ident[:])
                    xT = tpp.tile([128, 128], F32, tag='xTs')
                    nc.scalar.copy(xT[:], xT_ps[:])
                    nc.tensor.matmul(q_ps[h][:, jj * K:(jj + 1) * K],
                                     lhsT=xT[:], rhs=wT[:], start=True, stop=True)
                    wTr_ps = pst.tile([128, 128], F32, tag='wTr')
                    nc.tensor.transpose(out=wTr_ps[:], in_=wrows[:, j, :],
                                        identity=ident[:])
                    wTr = tpp.tile([128, 128], F32, tag='wTrs')
                    nc.scalar.copy(wTr[:], wTr_ps[:])
                    nc.tensor.matmul(p_ps[h][:, jj * K:(jj + 1) * K],
                                     lhsT=wTr[:], rhs=wT[:], start=True, stop=True)

                # softmax(q) -> y ; softmax(pl) -> prior ; argmax(q+g) -> kstar
                eq = midp.tile([128, N_SUB, K], F32, tag='eq')
                ep = midp.tile([128, N_SUB, K], F32, tag='ep')
                tq = midp.tile([128, N_SUB, K], F32, tag='tq')
                for h in range(2):
                    sl = slice(h * 8, (h + 1) * 8)
                    nc.scalar.activation(
                        eq[:, sl, :].rearrange("p a b -> p (a b)"), q_ps[h][:],
                        mybir.ActivationFunctionType.Exp)
                    nc.scalar.activation(
                        ep[:, sl, :].rearrange("p a b -> p (a b)"), p_ps[h][:],
                        mybir.ActivationFunctionType.Exp)
                    nc.vector.tensor_add(
                        tq[:, sl, :].rearrange("p a b -> p (a b)"), q_ps[h][:],
                        gtile[:, sl, :].rearrange("p a b -> p (a b)"))

                sq = smp.tile([128, N_SUB], F32, tag='sq')
                nc.vector.tensor_reduce(sq[:], eq[:], axis=mybir.AxisListType.X,
                                        op=mybir.AluOpType.add)
                rq = smp.tile([128, N_SUB], F32, tag='rq')
                nc.vector.reciprocal(rq[:], sq[:])
                ytile = midp.tile([128, N_SUB, K], F32, tag='yt')
                nc.vector.tensor_mul(ytile[:], eq[:],
                                     rq[:].rearrange("p a -> p a ()").to_broadcast([128, N_SUB, K]))
                nc.sync.dma_start(chunk_view(y_out, base),
                                  ytile[:].rearrange("p a b -> p (a b)"))

                sp = smp.tile([128, N_SUB], F32, tag='sp')
                nc.vector.tensor_reduce(sp[:], ep[:], axis=mybir.AxisListType.X,
                                        op=mybir.AluOpType.add)
                rp = smp.tile([128, N_SUB], F32, tag='rp')
                nc.vector.reciprocal(rp[:], sp[:])
                ptile = midp.tile([128, N_SUB, K], F32, tag='pt')
                nc.vector.tensor_mul(ptile[:], ep[:],
                                     rp[:].rearrange("p a -> p a ()").to_broadcast([128, N_SUB, K]))
                nc.sync.dma_start(chunk_view(p_out, base),
                                  ptile[:].rearrange("p a b -> p (a b)"))

                # argmax via grouped max + first-match reverse-iota trick
                m1 = smp.tile([128, N_SUB], F32, tag='m1')
                nc.vector.tensor_reduce(m1[:], tq[:], axis=mybir.AxisListType.X,
                                        op=mybir.AluOpType.max)
                eqm = midp.tile([128, N_SUB, K], F32, tag='eqm')
                nc.vector.tensor_tensor(
                    out=eqm[:], in0=tq[:],
                    in1=m1[:].rearrange("p a -> p a ()").to_broadcast([128, N_SUB, K]),
                    op=mybir.AluOpType.is_equal)
                sel = midp.tile([128, N_SUB, K], F32, tag='sel')
                nc.vector.tensor_mul(sel[:], eqm[:], iota_rev[:])
                m2 = smp.tile([128, N_SUB], F32, tag='m2')
                nc.vector.tensor_reduce(m2[:], sel[:], axis=mybir.AxisListType.X,
                                        op=mybir.AluOpType.max)
                kstar = smp.tile([128, N_SUB], F32, tag='kstar')
                nc.vector.tensor_scalar(out=kstar[:], in0=m2[:],
                                        scalar1=-1.0, scalar2=float(K),
                                        op0=mybir.AluOpType.mult,
                                        op1=mybir.AluOpType.add)

                # offsets: pos = c*64 + k*, neg = neg*64 + k*
                cf = smp.tile([128, N_SUB], F32, tag='cf')
                nc.vector.tensor_copy(cf[:], idxc[:])
                of_pos = smp.tile([128, N_SUB], F32, tag='ofp')
                nc.vector.tensor_scalar(out=of_pos[:], in0=cf[:],
                                        scalar1=float(K), scalar2=None,
                                        op0=mybir.AluOpType.mult)
                nc.vector.tensor_add(of_pos[:], of_pos[:], kstar[:])
                oi_pos = smp.tile([128, N_SUB], I32, tag='oip')
                nc.vector.tensor_copy(oi_pos[:], of_pos[:])

                nf = smp.tile([128, N_SUB, N_NEG], F32, tag='nf')
                nc.vector.tensor_copy(
                    nf[:].rearrange("p a b -> p (a b)"),
                    idxn[:].rearrange("p a b -> p (a b)"))
                of_neg = smp.tile([128, N_SUB, N_NEG], F32, tag='ofn')
                nc.vector.tensor_scalar(
                    out=of_neg[:].rearrange("p a b -> p (a b)"),
                    in0=nf[:].rearrange("p a b -> p (a b)"),
                    scalar1=float(K), scalar2=None, op0=mybir.AluOpType.mult)
                nc.vector.tensor_tensor(
                    out=of_neg[:], in0=of_neg[:],
                    in1=kstar[:].rearrange("p a -> p a ()").to_broadcast([128, N_SUB, N_NEG]),
                    op=mybir.AluOpType.add)
                oi_neg = smp.tile([128, N_SUB, N_NEG], I32, tag='oin')
                nc.vector.tensor_copy(
                    oi_neg[:].rearrange("p a b -> p (a b)"),
                    of_neg[:].rearrange("p a b -> p (a b)"))

                # phase B: element gathers of softplus tables
                scores = midp.tile([128, N_SUB, N_NEG + 1], F32, tag='sc')
                for j in range(N_SUB):
                    nc.gpsimd.indirect_dma_start(
                        out=scores[:, j, 0:1], out_offset=None, in_=gp_flat[:],
                        in_offset=bass.IndirectOffsetOnAxis(
                            ap=oi_pos[:, j:j + 1], axis=0))
                    for n in range(N_NEG):
                        nc.gpsimd.indirect_dma_start(
                            out=scores[:, j, n + 1:n + 2], out_offset=None,
                            in_=gn_flat[:],
                            in_offset=bass.IndirectOffsetOnAxis(
                                ap=oi_neg[:, j, n:n + 1], axis=0))
                csum = smp.tile([128, 1], F32, tag='csum')
                nc.vector.tensor_reduce(
                    csum[:], scores[:].rearrange("p a b -> p (a b)"),
                    axis=mybir.AxisListType.X, op=mybir.AluOpType.add)
                nc.vector.tensor_add(acc[:], acc[:], csum[:])

            nc.sync.dma_start(loss_out[:], acc[:])
    nc.finalize()
    return nc


# --------------------------------------------------------------------------- #
# host wrapper
# --------------------------------------------------------------------------- #

def kernel(w, c, neg, temp, gumbel_noise, node_emb, ctx_emb, community_w):
    w = np.ascontiguousarray(np.asarray(w, dtype=np.int64).astype(np.int32))
    c = np.ascontiguousarray(np.asarray(c, dtype=np.int64).astype(np.int32))
    neg = np.ascontiguousarray(np.asarray(neg, dtype=np.int64).astype(np.int32))
    gumbel = np.ascontiguousarray(np.asarray(gumbel_noise, dtype=np.float32))
    node = np.ascontiguousarray(np.asarray(node_emb, dtype=np.float32))
    ctx = np.ascontiguousarray(np.asarray(ctx_emb, dtype=np.float32))
    cw = np.ascontiguousarray(np.asarray(community_w, dtype=np.float32))
    tval = float(np.asarray(temp))
    assert tval > 0, "temp must be > 0 (argmax invariance)"

    rows = SIZE // N_CORES
    if 'l1' not in _COMPILED:
        _COMPILED['l1'] = build_l1(rows)
    res1 = _run_spmd(
        _COMPILED['l1'],
        [{'ctx': ctx[i * rows:(i + 1) * rows], 'cw': cw} for i in range(N_CORES)])
    gp = np.concatenate([r['gp'] for r in res1.results], axis=0)
    gn = np.concatenate([r['gn'] for r in res1.results], axis=0)

    # host relayout of index arrays: batch = core*B_CORE + ch*CHUNK + j*128 + p
    def relay(a):
        # a: [B] or [B, n] -> per core [N_CHUNK, 128, N_SUB(, n)]
        a2 = a.reshape(N_CORES, N_CHUNK, N_SUB, 128, *a.shape[1:])
        return np.ascontiguousarray(np.moveaxis(a2, 3, 2))

    wl, cl, negl = relay(w), relay(c), relay(neg)

    if 'l2' not in _COMPILED:
        _COMPILED['l2'] = build_l2()
    in_maps = []
    for i in range(N_CORES):
        in_maps.append({
            'node': node, 'gp': gp, 'gn': gn, 'cw': cw,
            'wl': wl[i], 'cl': cl[i], 'negl': negl[i],
            'gum': gumbel[i * B_CORE:(i + 1) * B_CORE],
        })
    res2 = _run_spmd(_COMPILED['l2'], in_maps)

    y = np.concatenate([r['y_out'] for r in res2.results], axis=0)
    prior = np.concatenate([r['p_out'] for r in res2.results], axis=0)
    loss = np.float32(sum(float(r['loss_out'].sum()) for r in res2.results) / B)
    return loss, y, prior


# revision 9
# speedup vs baseline: 1.3088x; 1.0335x over previous
"""Trainium2 Bass kernel for nn_GCNModelGumbel (gumbel-softmax skip-gram loss).

Math (matching reference.py, with z = stop_gradient(y_hard - y_soft) + y_soft
== y_hard numerically == onehot(argmax(q + gumbel)) for temp > 0):

  q[b]      = (node_emb[w_b] * node_emb[c_b]) @ W^T          [B, 64]
  y         = softmax(q)                                      (output 2)
  prior     = softmax(node_emb[w_b] @ W^T)                    (output 3)
  k*_b      = argmax(q + gumbel)
  loss      = mean_b[ sp(-proj[c_b, k*]) + 0.2 * sum_n sp(proj[neg_bn, k*]) ]
  where proj = ctx_emb @ W^T and sp = softplus                (output 1)

Two SPMD launches on 8 cores:
  L1: each core computes a 1/8 slice of Gp = sp(-proj), Gn2 = 0.2*sp(proj).
  L2: data-parallel over batch; per 2048-batch chunk: indirect row-gathers of
      node_emb for w/c, on-chip matmuls + softmaxes + argmax, then indirect
      element gathers of Gp/Gn2 at flat offsets idx*64 + k*.
"""
import sys
sys.path.insert(0, '/opt/trn_rl_repo')

import numpy as np

import concourse.bacc as bacc
import concourse.bass as bass
import concourse.mybir as mybir
import concourse.tile as tile
from concourse.masks import make_identity

SIZE = 100000
D = 128
K = 64
B = 131072
N_NEG = 5
N_CORES = 8
B_CORE = B // N_CORES            # 16384
CHUNK = 2048                     # batches per chunk
N_CHUNK = B_CORE // CHUNK        # 8
N_SUB = CHUNK // 128             # 16 subtiles per chunk

F32 = mybir.dt.float32
I32 = mybir.dt.int32

_COMPILED = {}


def _run_spmd(nc, in_maps):
    from concourse.bass_utils import run_bass_kernel_spmd
    return run_bass_kernel_spmd(nc, in_maps, core_ids=list(range(N_CORES)))


# --------------------------------------------------------------------------- #
# Launch 1: per-core slice of Gp / Gn2 tables
# --------------------------------------------------------------------------- #

def build_l1(rows):
    """rows = number of ctx_emb rows this core handles (SIZE/8 = 12500)."""
    nc = bacc.Bacc(None, target_bir_lowering=False)
    ctx = nc.dram_tensor('ctx', [rows, D], F32, kind='ExternalInput')
    cw = nc.dram_tensor('cw', [K, D], F32, kind='ExternalInput')
    gp = nc.dram_tensor('gp', [K, rows], F32, kind='ExternalOutput')
    gn = nc.dram_tensor('gn', [K, rows], F32, kind='ExternalOutput')

    GRP = 4  # 128-row tiles per group
    with tile.TileContext(nc) as tc:
        with tc.tile_pool(name='const', bufs=1) as cpool, \
             tc.tile_pool(name='work', bufs=4) as pool, \
             tc.tile_pool(name='tps', bufs=4) as tpool, \
             tc.tile_pool(name='ps', bufs=3, space='PSUM') as psum, \
             tc.tile_pool(name='ps2', bufs=3, space='PSUM') as psum2:
            ident = cpool.tile([128, 128], F32)
            make_identity(nc, ident[:])
            cw_t = cpool.tile([K, D], F32)
            nc.sync.dma_start(cw_t[:], cw[:])
            wT_ps = psum.tile([128, GRP * 128], F32, tag='xT')
            nc.tensor.transpose(out=wT_ps[:, :K], in_=cw_t[:], identity=ident[:K, :K])
            wT = cpool.tile([128, K], F32)
            nc.vector.tensor_copy(wT[:], wT_ps[:, :K])

            r0 = 0
            while r0 < rows:
                gsz = min(GRP * 128, rows - r0)
                nt = (gsz + 127) // 128
                rowst = pool.tile([128, GRP, D], F32, tag='rows')
                full = (gsz == GRP * 128)
                if full:
                    nc.sync.dma_start(
                        rowst[:].rearrange("p a b -> p (a b)"),
                        bass.AP(ctx, r0 * D, [[D, 128], [128 * D, GRP], [1, D]]))
                # all transposes of the group into one PSUM bank, one copy out
                xT_ps = psum.tile([128, GRP * 128], F32, tag='xT')
                for t in range(nt):
                    p = min(128, gsz - t * 128)
                    if not full:
                        rt = tpool.tile([128, D], F32, tag='rrow')
                        nc.sync.dma_start(rt[:p, :], ctx[r0 + t * 128:r0 + t * 128 + p, :])
                        src_ap = rt[:p, :]
                    else:
                        src_ap = rowst[:, t, :]
                    nc.tensor.transpose(out=xT_ps[:, t * 128:t * 128 + p],
                                        in_=src_ap, identity=ident[:p, :p])
                xT = tpool.tile([128, GRP * 128], F32, tag='xTs')
                nc.vector.tensor_copy(xT[:, :nt * 128], xT_ps[:, :nt * 128])
                # one matmul: projT [K, nt*128] = wT.T @ xT
                pr_ps = psum2.tile([K, GRP * 128], F32, tag='proj')
                nc.tensor.matmul(pr_ps[:, :nt * 128], lhsT=wT[:],
                                 rhs=xT[:, :nt * 128], start=True, stop=True)
                w_ = nt * 128
                e1 = pool.tile([K, GRP * 128], F32, tag='e1')
                nc.scalar.activation(e1[:, :w_], pr_ps[:, :w_],
                                     mybir.ActivationFunctionType.Exp, scale=-1.0)
                g1 = pool.tile([K, GRP * 128], F32, tag='g1')
                nc.scalar.activation(g1[:, :w_], e1[:, :w_],
                                     mybir.ActivationFunctionType.Ln, bias=1.0)
                nc.sync.dma_start(gp[:, r0:r0 + gsz], g1[:, :w_] if full else g1[:, :gsz])
                e2 = pool.tile([K, GRP * 128], F32, tag='e2')
                nc.scalar.activation(e2[:, :w_], pr_ps[:, :w_],
                                     mybir.ActivationFunctionType.Exp)
                g2 = pool.tile([K, GRP * 128], F32, tag='g2')
                nc.scalar.activation(g2[:, :w_], e2[:, :w_],
                                     mybir.ActivationFunctionType.Ln, bias=1.0)
                g2s = pool.tile([K, GRP * 128], F32, tag='g2s')
                nc.vector.tensor_scalar_mul(g2s[:, :w_], g2[:, :w_], 1.0 / N_NEG)
                nc.sync.dma_start(gn[:, r0:r0 + gsz], g2s[:, :w_] if full else g2s[:, :gsz])
                r0 += gsz
    nc.finalize()
    return nc


# --------------------------------------------------------------------------- #
# Launch 2: main kernel (per-core batch shard)
# --------------------------------------------------------------------------- #

def build_l2():
    nc = bacc.Bacc(None, target_bir_lowering=False)
    node = nc.dram_tensor('node', [SIZE, D], F32, kind='ExternalInput')
    gp = nc.dram_tensor('gp', [K, SIZE], F32, kind='ExternalInput')
    gn = nc.dram_tensor('gn', [K, SIZE], F32, kind='ExternalInput')
    cw = nc.dram_tensor('cw', [K, D], F32, kind='ExternalInput')
    # host-relaid index tensors: [nchunk, 128, ...] with batch = base + j*128 + p
    wl = nc.dram_tensor('wl', [N_CHUNK, 128, N_SUB], I32, kind='ExternalInput')
    cl = nc.dram_tensor('cl', [N_CHUNK, 128, N_SUB], I32, kind='ExternalInput')
    negl = nc.dram_tensor('negl', [N_CHUNK, 128, N_SUB, N_NEG], I32,
                          kind='ExternalInput')
    gum = nc.dram_tensor('gum', [B_CORE, K], F32, kind='ExternalInput')
    y_out = nc.dram_tensor('y_out', [B_CORE, K], F32, kind='ExternalOutput')
    p_out = nc.dram_tensor('p_out', [B_CORE, K], F32, kind='ExternalOutput')
    loss_out = nc.dram_tensor('loss_out', [128, 1], F32, kind='ExternalOutput')


    def chunk_view(t, base):
        # [B_CORE, K] dram tensor viewed as [p=128, a=N_SUB, b=K] for batch
        # row = base + a*128 + p
        return bass.AP(t, base * K, [[K, 128], [128 * K, N_SUB], [1, K]])
    gp_flat = gp.rearrange("k v -> (k v) ()")
    gn_flat = gn.rearrange("k v -> (k v) ()")

    with tile.TileContext(nc) as tc:
        with tc.tile_pool(name='const', bufs=1) as cpool, \
             tc.tile_pool(name='io', bufs=2) as iop, \
             tc.tile_pool(name='big', bufs=2) as bigp, \
             tc.tile_pool(name='mid', bufs=2) as midp, \
             tc.tile_pool(name='sm', bufs=3) as smp, \
             tc.tile_pool(name='tp', bufs=4) as tpp, \
             tc.tile_pool(name='psq', bufs=1, space='PSUM') as psq, \
             tc.tile_pool(name='psp', bufs=1, space='PSUM') as psp, \
             tc.tile_pool(name='pst', bufs=2, space='PSUM') as pst:
            ident = cpool.tile([128, 128], F32)
            make_identity(nc, ident[:])
            cw_t = cpool.tile([K, D], F32)
            nc.sync.dma_start(cw_t[:], cw[:])
            wT_ps = pst.tile([128, K], F32, tag='xT')
            nc.tensor.transpose(out=wT_ps[:], in_=cw_t[:], identity=ident[:K, :K])
            wT = cpool.tile([128, K], F32)
            nc.vector.tensor_copy(wT[:], wT_ps[:])
            # iota along K repeated per subtile: [128, N_SUB, K], value = k
            iota_i = cpool.tile([128, N_SUB * K], I32)
            nc.gpsimd.iota(iota_i[:], pattern=[[0, N_SUB], [1, K]],
                           channel_multiplier=0)
            iota_f = cpool.tile([128, N_SUB, K], F32)
            nc.vector.tensor_copy(iota_f[:].rearrange("p a b -> p (a b)"), iota_i[:])
            # reversed iota 64-k (for first-argmax tie-break): value = K - k
            iota_rev = cpool.tile([128, N_SUB, K], F32)
            nc.vector.tensor_scalar(
                out=iota_rev[:].rearrange("p a b -> p (a b)"),
                in0=iota_f[:].rearrange("p a b -> p (a b)"),
                scalar1=-1.0, scalar2=float(K),
                op0=mybir.AluOpType.mult, op1=mybir.AluOpType.add)
            acc = cpool.tile([128, 1], F32)
            nc.vector.memset(acc[:], 0.0)

            for ch in range(N_CHUNK):
                base = ch * CHUNK
                idxw = iop.tile([128, N_SUB], I32, tag='idxw')
                nc.sync.dma_start(idxw[:], wl[ch])
                idxc = iop.tile([128, N_SUB], I32, tag='idxc')
                nc.sync.dma_start(idxc[:], cl[ch])
                idxn = iop.tile([128, N_SUB, N_NEG], I32, tag='idxn')
                nc.sync.dma_start(idxn[:], negl[ch])
                gtile = bigp.tile([128, N_SUB, K], F32, tag='gum')
                nc.sync.dma_start(
                    gtile[:].rearrange("p a b -> p (a b)"), chunk_view(gum, base))

                wrows = bigp.tile([128, N_SUB, D], F32, tag='wrows')
                crows = bigp.tile([128, N_SUB, D], F32, tag='crows')
                for j in range(N_SUB):
                    nc.gpsimd.indirect_dma_start(
                        out=wrows[:, j, :], out_offset=None, in_=node[:],
                        in_offset=bass.IndirectOffsetOnAxis(
                            ap=idxw[:, j:j + 1], axis=0))
                    nc.gpsimd.indirect_dma_start(
                        out=crows[:, j, :], out_offset=None, in_=node[:],
                        in_offset=bass.IndirectOffsetOnAxis(
                            ap=idxc[:, j:j + 1], axis=0))
                xprod = bigp.tile([128, N_SUB, D], F32, tag='xprod')
                nc.vector.tensor_mul(
                    xprod[:].rearrange("p a b -> p (a b)"),
                    wrows[:].rearrange("p a b -> p (a b)"),
                    crows[:].rearrange("p a b -> p (a b)"))

                # matmuls: q = x @ W^T, pl = w_e @ W^T, PSUM-packed 8 subtiles/bank
                q_ps0 = psq.tile([128, 8 * K], F32, tag='q0')
                q_ps1 = psq.tile([128, 8 * K], F32, tag='q1')
                p_ps0 = psp.tile([128, 8 * K], F32, tag='p0')
                p_ps1 = psp.tile([128, 8 * K], F32, tag='p1')
                q_ps = [q_ps0, q_ps1]
                p_ps = [p_ps0, p_ps1]
                for j in range(N_SUB):
                    h, jj = j // 8, j % 8
                    xT_ps = pst.tile([128, 128], F32, tag='xT')
                    nc.tensor.transpose(out=xT_ps[:], in_=xprod[:, j, :],
                                        identity=ident[:])
                    xT = tpp.tile([128, 128], F32, tag='xTs')
                    nc.scalar.copy(xT[:], xT_ps[:])
                    nc.tensor.matmul(q_ps[h][:, jj * K:(jj + 1) * K],
                                     lhsT=xT[:], rhs=wT[:], start=True, stop=True)
                    wTr_ps = pst.tile([128, 128], F32, tag='wTr')
                    nc.tensor.transpose(out=wTr_ps[:], in_=wrows[:, j, :],
                                        identity=ident[:])
                    wTr = tpp.tile([128, 128], F32, tag='wTrs')
                    nc.scalar.copy(wTr[:], wTr_ps[:])
                    nc.tensor.matmul(p_ps[h][:, jj * K:(jj + 1) * K],
                                     lhsT=wTr[:], rhs=wT[:], start=True, stop=True)

                # softmax(q) -> y ; softmax(pl) -> prior ; argmax(q+g) -> kstar
                eq = midp.tile([128, N_SUB, K], F32, tag='eq')
                ep = midp.tile([128, N_SUB, K], F32, tag='ep')
                tq = midp.tile([128, N_SUB, K], F32, tag='tq')
                for h in range(2):
                    sl = slice(h * 8, (h + 1) * 8)
                    nc.scalar.activation(
                        eq[:, sl, :].rearrange("p a b -> p (a b)"), q_ps[h][:],
                        mybir.ActivationFunctionType.Exp)
                    nc.scalar.activation(
                        ep[:, sl, :].rearrange("p a b -> p (a b)"), p_ps[h][:],
                        mybir.ActivationFunctionType.Exp)
                    nc.vector.tensor_add(
                        tq[:, sl, :].rearrange("p a b -> p (a b)"), q_ps[h][:],
                        gtile[:, sl, :].rearrange("p a b -> p (a b)"))

                sq = smp.tile([128, N_SUB], F32, tag='sq')
                nc.vector.tensor_reduce(sq[:], eq[:], axis=mybir.AxisListType.X,
                                        op=mybir.AluOpType.add)
                rq = smp.tile([128, N_SUB], F32, tag='rq')
                nc.vector.reciprocal(rq[:], sq[:])
                ytile = midp.tile([128, N_SUB, K], F32, tag='yt')
                nc.vector.tensor_mul(ytile[:], eq[:],
                                     rq[:].rearrange("p a -> p a ()").to_broadcast([128, N_SUB, K]))
                nc.sync.dma_start(chunk_view(y_out, base),
                                  ytile[:].rearrange("p a b -> p (a b)"))

                sp = smp.tile([128, N_SUB], F32, tag='sp')
                nc.vector.tensor_reduce(sp[:], ep[:], axis=mybir.AxisListType.X,
                                        op=mybir.AluOpType.add)
                rp = smp.tile([128, N_SUB], F32, tag='rp')
                nc.vector.reciprocal(rp[:], sp[:])
                ptile = midp.tile([128, N_SUB, K], F32, tag='pt')
                nc.vector.tensor_mul(ptile[:], ep[:],
                                     rp[:].rearrange("p a -> p a ()").to_broadcast([128, N_SUB, K]))
                nc.sync.dma_start(chunk_view(p_out, base),
                                  ptile[:].rearrange("p a b -> p (a b)"))

                # argmax via grouped max + first-match reverse-iota trick
                m1 = smp.tile([128, N_SUB], F32, tag='m1')
                nc.vector.tensor_reduce(m1[:], tq[:], axis=mybir.AxisListType.X,
                                        op=mybir.AluOpType.max)
                eqm = midp.tile([128, N_SUB, K], F32, tag='eqm')
                nc.vector.tensor_tensor(
                    out=eqm[:], in0=tq[:],
                    in1=m1[:].rearrange("p a -> p a ()").to_broadcast([128, N_SUB, K]),
                    op=mybir.AluOpType.is_equal)
                sel = midp.tile([128, N_SUB, K], F32, tag='sel')
                nc.vector.tensor_mul(sel[:], eqm[:], iota_rev[:])
                m2 = smp.tile([128, N_SUB], F32, tag='m2')
                nc.vector.tensor_reduce(m2[:], sel[:], axis=mybir.AxisListType.X,
                                        op=mybir.AluOpType.max)
                kstar = smp.tile([128, N_SUB], F32, tag='kstar')
                nc.vector.tensor_scalar(out=kstar[:], in0=m2[:],
                                        scalar1=-1.0, scalar2=float(K),
                                        op0=mybir.AluOpType.mult,
                                        op1=mybir.AluOpType.add)

                # offsets: pos = c*64 + k*, neg = neg*64 + k*
                ks2 = smp.tile([128, N_SUB], F32, tag='ks2')
                nc.vector.tensor_scalar(out=ks2[:], in0=kstar[:],
                                        scalar1=float(SIZE), scalar2=None,
                                        op0=mybir.AluOpType.mult)
                cf = smp.tile([128, N_SUB], F32, tag='cf')
                nc.vector.tensor_copy(cf[:], idxc[:])
                of_pos = smp.tile([128, N_SUB], F32, tag='ofp')
                nc.vector.tensor_add(of_pos[:], cf[:], ks2[:])
                oi_pos = smp.tile([128, N_SUB], I32, tag='oip')
                nc.vector.tensor_copy(oi_pos[:], of_pos[:])

                nf = smp.tile([128, N_SUB, N_NEG], F32, tag='nf')
                nc.vector.tensor_copy(
                    nf[:].rearrange("p a b -> p (a b)"),
                    idxn[:].rearrange("p a b -> p (a b)"))
                of_neg = smp.tile([128, N_SUB, N_NEG], F32, tag='ofn')
                nc.vector.tensor_tensor(
                    out=of_neg[:], in0=nf[:],
                    in1=ks2[:].rearrange("p a -> p a ()").to_broadcast([128, N_SUB, N_NEG]),
                    op=mybir.AluOpType.add)
                oi_neg = smp.tile([128, N_SUB, N_NEG], I32, tag='oin')
                nc.vector.tensor_copy(
                    oi_neg[:].rearrange("p a b -> p (a b)"),
                    of_neg[:].rearrange("p a b -> p (a b)"))

                # phase B: element gathers of softplus tables
                scores = midp.tile([128, N_SUB, N_NEG + 1], F32, tag='sc')
                for j in range(N_SUB):
                    nc.gpsimd.indirect_dma_start(
                        out=scores[:, j, 0:1], out_offset=None, in_=gp_flat[:],
                        in_offset=bass.IndirectOffsetOnAxis(
                            ap=oi_pos[:, j:j + 1], axis=0))
                    for n in range(N_NEG):
                        nc.gpsimd.indirect_dma_start(
                            out=scores[:, j, n + 1:n + 2], out_offset=None,
                            in_=gn_flat[:],
                            in_offset=bass.IndirectOffsetOnAxis(
                                ap=oi_neg[:, j, n:n + 1], axis=0))
                csum = smp.tile([128, 1], F32, tag='csum')
                nc.vector.tensor_reduce(
                    csum[:], scores[:].rearrange("p a b -> p (a b)"),
                    axis=mybir.AxisListType.X, op=mybir.AluOpType.add)
                nc.vector.tensor_add(acc[:], acc[:], csum[:])

            nc.sync.dma_start(loss_out[:], acc[:])
    nc.finalize()
    return nc


# --------------------------------------------------------------------------- #
# host wrapper
# --------------------------------------------------------------------------- #

def kernel(w, c, neg, temp, gumbel_noise, node_emb, ctx_emb, community_w):
    w = np.ascontiguousarray(np.asarray(w, dtype=np.int64).astype(np.int32))
    c = np.ascontiguousarray(np.asarray(c, dtype=np.int64).astype(np.int32))
    neg = np.ascontiguousarray(np.asarray(neg, dtype=np.int64).astype(np.int32))
    gumbel = np.ascontiguousarray(np.asarray(gumbel_noise, dtype=np.float32))
    node = np.ascontiguousarray(np.asarray(node_emb, dtype=np.float32))
    ctx = np.ascontiguousarray(np.asarray(ctx_emb, dtype=np.float32))
    cw = np.ascontiguousarray(np.asarray(community_w, dtype=np.float32))
    tval = float(np.asarray(temp))
    assert tval > 0, "temp must be > 0 (argmax invariance)"

    rows = SIZE // N_CORES
    if 'l1' not in _COMPILED:
        _COMPILED['l1'] = build_l1(rows)
    res1 = _run_spmd(
        _COMPILED['l1'],
        [{'ctx': ctx[i * rows:(i + 1) * rows], 'cw': cw} for i in range(N_CORES)])
    gp = np.ascontiguousarray(np.concatenate([r['gp'] for r in res1.results], axis=1))
    gn = np.ascontiguousarray(np.concatenate([r['gn'] for r in res1.results], axis=1))

    # host relayout of index arrays: batch = core*B_CORE + ch*CHUNK + j*128 + p
    def relay(a):
        # a: [B] or [B, n] -> per core [N_CHUNK, 128, N_SUB(, n)]
        a2 = a.reshape(N_CORES, N_CHUNK, N_SUB, 128, *a.shape[1:])
        return np.ascontiguousarray(np.moveaxis(a2, 3, 2))

    wl, cl, negl = relay(w), relay(c), relay(neg)

    if 'l2' not in _COMPILED:
        _COMPILED['l2'] = build_l2()
    in_maps = []
    for i in range(N_CORES):
        in_maps.append({
            'node': node, 'gp': gp, 'gn': gn, 'cw': cw,
            'wl': wl[i], 'cl': cl[i], 'negl': negl[i],
            'gum': gumbel[i * B_CORE:(i + 1) * B_CORE],
        })
    res2 = _run_spmd(_COMPILED['l2'], in_maps)

    y = np.concatenate([r['y_out'] for r in res2.results], axis=0)
    prior = np.concatenate([r['p_out'] for r in res2.results], axis=0)
    loss = np.float32(sum(float(r['loss_out'].sum()) for r in res2.results) / B)
    return loss, y, prior


# revision 10
# speedup vs baseline: 1.3307x; 1.0167x over previous
"""Trainium2 Bass kernel for nn_GCNModelGumbel (gumbel-softmax skip-gram loss).

Math (matching reference.py, with z = stop_gradient(y_hard - y_soft) + y_soft
== y_hard numerically == onehot(argmax(q + gumbel)) for temp > 0):

  q[b]      = (node_emb[w_b] * node_emb[c_b]) @ W^T          [B, 64]
  y         = softmax(q)                                      (output 2)
  prior     = softmax(node_emb[w_b] @ W^T)                    (output 3)
  k*_b      = argmax(q + gumbel)
  loss      = mean_b[ sp(-proj[c_b, k*]) + 0.2 * sum_n sp(proj[neg_bn, k*]) ]
  where proj = ctx_emb @ W^T and sp = softplus                (output 1)

Two SPMD launches on 8 cores:
  L1: each core computes a 1/8 slice of Gp = sp(-proj), Gn2 = 0.2*sp(proj).
  L2: data-parallel over batch; per 2048-batch chunk: indirect row-gathers of
      node_emb for w/c, on-chip matmuls + softmaxes + argmax, then indirect
      element gathers of Gp/Gn2 at flat offsets idx*64 + k*.
"""
import sys
sys.path.insert(0, '/opt/trn_rl_repo')

import numpy as np

import concourse.bacc as bacc
import concourse.bass as bass
import concourse.mybir as mybir
import concourse.tile as tile
from concourse.masks import make_identity

SIZE = 100000
D = 128
K = 64
B = 131072
N_NEG = 5
N_CORES = 8
B_CORE = B // N_CORES            # 16384
CHUNK = 2048                     # batches per chunk
N_CHUNK = B_CORE // CHUNK        # 8
N_SUB = CHUNK // 128             # 16 subtiles per chunk

F32 = mybir.dt.float32
I32 = mybir.dt.int32

_COMPILED = {}


def _run_spmd(nc, in_maps):
    from concourse.bass_utils import run_bass_kernel_spmd
    return run_bass_kernel_spmd(nc, in_maps, core_ids=list(range(N_CORES)))


# --------------------------------------------------------------------------- #
# Launch 1: per-core slice of Gp / Gn2 tables
# --------------------------------------------------------------------------- #

def build_l1(rows):
    """rows = number of ctx_emb rows this core handles (SIZE/8 = 12500)."""
    nc = bacc.Bacc(None, target_bir_lowering=False)
    ctx = nc.dram_tensor('ctx', [rows, D], F32, kind='ExternalInput')
    cw = nc.dram_tensor('cw', [K, D], F32, kind='ExternalInput')
    gp = nc.dram_tensor('gp', [K, rows], F32, kind='ExternalOutput')
    gn = nc.dram_tensor('gn', [K, rows], F32, kind='ExternalOutput')

    GRP = 4  # 128-row tiles per group
    with tile.TileContext(nc) as tc:
        with tc.tile_pool(name='const', bufs=1) as cpool, \
             tc.tile_pool(name='work', bufs=4) as pool, \
             tc.tile_pool(name='tps', bufs=4) as tpool, \
             tc.tile_pool(name='ps', bufs=3, space='PSUM') as psum, \
             tc.tile_pool(name='ps2', bufs=3, space='PSUM') as psum2:
            ident = cpool.tile([128, 128], F32)
            make_identity(nc, ident[:])
            cw_t = cpool.tile([K, D], F32)
            nc.sync.dma_start(cw_t[:], cw[:])
            wT_ps = psum.tile([128, GRP * 128], F32, tag='xT')
            nc.tensor.transpose(out=wT_ps[:, :K], in_=cw_t[:], identity=ident[:K, :K])
            wT = cpool.tile([128, K], F32)
            nc.vector.tensor_copy(wT[:], wT_ps[:, :K])

            r0 = 0
            while r0 < rows:
                gsz = min(GRP * 128, rows - r0)
                nt = (gsz + 127) // 128
                rowst = pool.tile([128, GRP, D], F32, tag='rows')
                full = (gsz == GRP * 128)
                if full:
                    nc.sync.dma_start(
                        rowst[:].rearrange("p a b -> p (a b)"),
                        bass.AP(ctx, r0 * D, [[D, 128], [128 * D, GRP], [1, D]]))
                # all transposes of the group into one PSUM bank, one copy out
                xT_ps = psum.tile([128, GRP * 128], F32, tag='xT')
                for t in range(nt):
                    p = min(128, gsz - t * 128)
                    if not full:
                        rt = tpool.tile([128, D], F32, tag='rrow')
                        nc.sync.dma_start(rt[:p, :], ctx[r0 + t * 128:r0 + t * 128 + p, :])
                        src_ap = rt[:p, :]
                    else:
                        src_ap = rowst[:, t, :]
                    nc.tensor.transpose(out=xT_ps[:, t * 128:t * 128 + p],
                                        in_=src_ap, identity=ident[:p, :p])
                xT = tpool.tile([128, GRP * 128], F32, tag='xTs')
                nc.vector.tensor_copy(xT[:, :nt * 128], xT_ps[:, :nt * 128])
                # one matmul: projT [K, nt*128] = wT.T @ xT
                pr_ps = psum2.tile([K, GRP * 128], F32, tag='proj')
                nc.tensor.matmul(pr_ps[:, :nt * 128], lhsT=wT[:],
                                 rhs=xT[:, :nt * 128], start=True, stop=True)
                w_ = nt * 128
                e1 = pool.tile([K, GRP * 128], F32, tag='e1')
                nc.scalar.activation(e1[:, :w_], pr_ps[:, :w_],
                                     mybir.ActivationFunctionType.Exp, scale=-1.0)
                g1 = pool.tile([K, GRP * 128], F32, tag='g1')
                nc.scalar.activation(g1[:, :w_], e1[:, :w_],
                                     mybir.ActivationFunctionType.Ln, bias=1.0)
                nc.sync.dma_start(gp[:, r0:r0 + gsz], g1[:, :w_] if full else g1[:, :gsz])
                # sp(x) = x + sp(-x): reuse g1 instead of a second Exp/Ln chain
                g2 = pool.tile([K, GRP * 128], F32, tag='g2')
                nc.vector.tensor_add(g2[:, :w_], pr_ps[:, :w_], g1[:, :w_])
                g2s = pool.tile([K, GRP * 128], F32, tag='g2s')
                nc.vector.tensor_scalar_mul(g2s[:, :w_], g2[:, :w_], 1.0 / N_NEG)
                nc.sync.dma_start(gn[:, r0:r0 + gsz], g2s[:, :w_] if full else g2s[:, :gsz])
                r0 += gsz
    nc.finalize()
    return nc


# --------------------------------------------------------------------------- #
# Launch 2: main kernel (per-core batch shard)
# --------------------------------------------------------------------------- #

def build_l2():
    nc = bacc.Bacc(None, target_bir_lowering=False)
    node = nc.dram_tensor('node', [SIZE, D], F32, kind='ExternalInput')
    gp = nc.dram_tensor('gp', [K, SIZE], F32, kind='ExternalInput')
    gn = nc.dram_tensor('gn', [K, SIZE], F32, kind='ExternalInput')
    cw = nc.dram_tensor('cw', [K, D], F32, kind='ExternalInput')
    # host-relaid index tensors: [nchunk, 128, ...] with batch = base + j*128 + p
    wl = nc.dram_tensor('wl', [N_CHUNK, 128, N_SUB], I32, kind='ExternalInput')
    cl = nc.dram_tensor('cl', [N_CHUNK, 128, N_SUB], I32, kind='ExternalInput')
    negl = nc.dram_tensor('negl', [N_CHUNK, 128, N_SUB, N_NEG], I32,
                          kind='ExternalInput')
    gum = nc.dram_tensor('gum', [B_CORE, K], F32, kind='ExternalInput')
    y_out = nc.dram_tensor('y_out', [B_CORE, K], F32, kind='ExternalOutput')
    p_out = nc.dram_tensor('p_out', [B_CORE, K], F32, kind='ExternalOutput')
    loss_out = nc.dram_tensor('loss_out', [128, 1], F32, kind='ExternalOutput')


    def chunk_view(t, base):
        # [B_CORE, K] dram tensor viewed as [p=128, a=N_SUB, b=K] for batch
        # row = base + a*128 + p
        return bass.AP(t, base * K, [[K, 128], [128 * K, N_SUB], [1, K]])
    gp_flat = gp.rearrange("k v -> (k v) ()")
    gn_flat = gn.rearrange("k v -> (k v) ()")

    with tile.TileContext(nc) as tc:
        with tc.tile_pool(name='const', bufs=1) as cpool, \
             tc.tile_pool(name='io', bufs=2) as iop, \
             tc.tile_pool(name='big', bufs=2) as bigp, \
             tc.tile_pool(name='mid', bufs=2) as midp, \
             tc.tile_pool(name='sm', bufs=3) as smp, \
             tc.tile_pool(name='tp', bufs=4) as tpp, \
             tc.tile_pool(name='psq', bufs=1, space='PSUM') as psq, \
             tc.tile_pool(name='psp', bufs=1, space='PSUM') as psp, \
             tc.tile_pool(name='pst', bufs=2, space='PSUM') as pst:
            ident = cpool.tile([128, 128], F32)
            make_identity(nc, ident[:])
            cw_t = cpool.tile([K, D], F32)
            nc.sync.dma_start(cw_t[:], cw[:])
            wT_ps = pst.tile([128, K], F32, tag='xT')
            nc.tensor.transpose(out=wT_ps[:], in_=cw_t[:], identity=ident[:K, :K])
            wT = cpool.tile([128, K], F32)
            nc.vector.tensor_copy(wT[:], wT_ps[:])
            # iota along K repeated per subtile: [128, N_SUB, K], value = k
            iota_i = cpool.tile([128, N_SUB * K], I32)
            nc.gpsimd.iota(iota_i[:], pattern=[[0, N_SUB], [1, K]],
                           channel_multiplier=0)
            iota_f = cpool.tile([128, N_SUB, K], F32)
            nc.vector.tensor_copy(iota_f[:].rearrange("p a b -> p (a b)"), iota_i[:])
            # reversed iota 64-k (for first-argmax tie-break): value = K - k
            iota_rev = cpool.tile([128, N_SUB, K], F32)
            nc.vector.tensor_scalar(
                out=iota_rev[:].rearrange("p a b -> p (a b)"),
                in0=iota_f[:].rearrange("p a b -> p (a b)"),
                scalar1=-1.0, scalar2=float(K),
                op0=mybir.AluOpType.mult, op1=mybir.AluOpType.add)
            acc = cpool.tile([128, 1], F32)
            nc.vector.memset(acc[:], 0.0)

            for ch in range(N_CHUNK):
                base = ch * CHUNK
                idxw = iop.tile([128, N_SUB], I32, tag='idxw')
                nc.sync.dma_start(idxw[:], wl[ch])
                idxc = iop.tile([128, N_SUB], I32, tag='idxc')
                nc.sync.dma_start(idxc[:], cl[ch])
                idxn = iop.tile([128, N_SUB, N_NEG], I32, tag='idxn')
                nc.sync.dma_start(idxn[:], negl[ch])
                gtile = bigp.tile([128, N_SUB, K], F32, tag='gum')
                nc.sync.dma_start(
                    gtile[:].rearrange("p a b -> p (a b)"), chunk_view(gum, base))

                wrows = bigp.tile([128, N_SUB, D], F32, tag='wrows')
                crows = bigp.tile([128, N_SUB, D], F32, tag='crows')
                for j in range(N_SUB):
                    nc.gpsimd.indirect_dma_start(
                        out=wrows[:, j, :], out_offset=None, in_=node[:],
                        in_offset=bass.IndirectOffsetOnAxis(
                            ap=idxw[:, j:j + 1], axis=0))
                    nc.gpsimd.indirect_dma_start(
                        out=crows[:, j, :], out_offset=None, in_=node[:],
                        in_offset=bass.IndirectOffsetOnAxis(
                            ap=idxc[:, j:j + 1], axis=0))
                xprod = bigp.tile([128, N_SUB, D], F32, tag='xprod')
                nc.vector.tensor_mul(
                    xprod[:].rearrange("p a b -> p (a b)"),
                    wrows[:].rearrange("p a b -> p (a b)"),
                    crows[:].rearrange("p a b -> p (a b)"))

                # matmuls: q = x @ W^T, pl = w_e @ W^T, PSUM-packed 8 subtiles/bank
                q_ps0 = psq.tile([128, 8 * K], F32, tag='q0')
                q_ps1 = psq.tile([128, 8 * K], F32, tag='q1')
                p_ps0 = psp.tile([128, 8 * K], F32, tag='p0')
                p_ps1 = psp.tile([128, 8 * K], F32, tag='p1')
                q_ps = [q_ps0, q_ps1]
                p_ps = [p_ps0, p_ps1]
                for j in range(N_SUB):
                    h, jj = j // 8, j % 8
                    xT_ps = pst.tile([128, 128], F32, tag='xT')
                    nc.tensor.transpose(out=xT_ps[:], in_=xprod[:, j, :],
                                        identity=ident[:])
                    xT = tpp.tile([128, 128], F32, tag='xTs')
                    nc.scalar.copy(xT[:], xT_ps[:])
                    nc.tensor.matmul(q_ps[h][:, jj * K:(jj + 1) * K],
                                     lhsT=xT[:], rhs=wT[:], start=True, stop=True)
                    wTr_ps = pst.tile([128, 128], F32, tag='wTr')
                    nc.tensor.transpose(out=wTr_ps[:], in_=wrows[:, j, :],
                                        identity=ident[:])
                    wTr = tpp.tile([128, 128], F32, tag='wTrs')
                    nc.scalar.copy(wTr[:], wTr_ps[:])
                    nc.tensor.matmul(p_ps[h][:, jj * K:(jj + 1) * K],
                                     lhsT=wTr[:], rhs=wT[:], start=True, stop=True)

                # softmax(q) -> y ; softmax(pl) -> prior ; argmax(q+g) -> kstar
                eq = midp.tile([128, N_SUB, K], F32, tag='eq')
                ep = midp.tile([128, N_SUB, K], F32, tag='ep')
                tq = midp.tile([128, N_SUB, K], F32, tag='tq')
                for h in range(2):
                    sl = slice(h * 8, (h + 1) * 8)
                    nc.scalar.activation(
                        eq[:, sl, :].rearrange("p a b -> p (a b)"), q_ps[h][:],
                        mybir.ActivationFunctionType.Exp)
                    nc.scalar.activation(
                        ep[:, sl, :].rearrange("p a b -> p (a b)"), p_ps[h][:],
                        mybir.ActivationFunctionType.Exp)
                    nc.vector.tensor_add(
                        tq[:, sl, :].rearrange("p a b -> p (a b)"), q_ps[h][:],
                        gtile[:, sl, :].rearrange("p a b -> p (a b)"))

                sq = smp.tile([128, N_SUB], F32, tag='sq')
                nc.vector.tensor_reduce(sq[:], eq[:], axis=mybir.AxisListType.X,
                                        op=mybir.AluOpType.add)
                rq = smp.tile([128, N_SUB], F32, tag='rq')
                nc.vector.reciprocal(rq[:], sq[:])
                ytile = midp.tile([128, N_SUB, K], F32, tag='yt')
                nc.vector.tensor_mul(ytile[:], eq[:],
                                     rq[:].rearrange("p a -> p a ()").to_broadcast([128, N_SUB, K]))
                nc.sync.dma_start(chunk_view(y_out, base),
                                  ytile[:].rearrange("p a b -> p (a b)"))

                sp = smp.tile([128, N_SUB], F32, tag='sp')
                nc.vector.tensor_reduce(sp[:], ep[:], axis=mybir.AxisListType.X,
                                        op=mybir.AluOpType.add)
                rp = smp.tile([128, N_SUB], F32, tag='rp')
                nc.vector.reciprocal(rp[:], sp[:])
                ptile = midp.tile([128, N_SUB, K], F32, tag='pt')
                nc.vector.tensor_mul(ptile[:], ep[:],
                                     rp[:].rearrange("p a -> p a ()").to_broadcast([128, N_SUB, K]))
                nc.sync.dma_start(chunk_view(p_out, base),
                                  ptile[:].rearrange("p a b -> p (a b)"))

                # argmax via grouped max + first-match reverse-iota trick
                m1 = smp.tile([128, N_SUB], F32, tag='m1')
                nc.vector.tensor_reduce(m1[:], tq[:], axis=mybir.AxisListType.X,
                                        op=mybir.AluOpType.max)
                eqm = midp.tile([128, N_SUB, K], F32, tag='eqm')
                nc.vector.tensor_tensor(
                    out=eqm[:], in0=tq[:],
                    in1=m1[:].rearrange("p a -> p a ()").to_broadcast([128, N_SUB, K]),
                    op=mybir.AluOpType.is_equal)
                sel = midp.tile([128, N_SUB, K], F32, tag='sel')
                nc.vector.tensor_mul(sel[:], eqm[:], iota_rev[:])
                m2 = smp.tile([128, N_SUB], F32, tag='m2')
                nc.vector.tensor_reduce(m2[:], sel[:], axis=mybir.AxisListType.X,
                                        op=mybir.AluOpType.max)
                kstar = smp.tile([128, N_SUB], F32, tag='kstar')
                nc.vector.tensor_scalar(out=kstar[:], in0=m2[:],
                                        scalar1=-1.0, scalar2=float(K),
                                        op0=mybir.AluOpType.mult,
                                        op1=mybir.AluOpType.add)

                # offsets: pos = c*64 + k*, neg = neg*64 + k*
                ks2 = smp.tile([128, N_SUB], F32, tag='ks2')
                nc.vector.tensor_scalar(out=ks2[:], in0=kstar[:],
                                        scalar1=float(SIZE), scalar2=None,
                                        op0=mybir.AluOpType.mult)
                cf = smp.tile([128, N_SUB], F32, tag='cf')
                nc.vector.tensor_copy(cf[:], idxc[:])
                of_pos = smp.tile([128, N_SUB], F32, tag='ofp')
                nc.vector.tensor_add(of_pos[:], cf[:], ks2[:])
                oi_pos = smp.tile([128, N_SUB], I32, tag='oip')
                nc.vector.tensor_copy(oi_pos[:], of_pos[:])

                nf = smp.tile([128, N_SUB, N_NEG], F32, tag='nf')
                nc.vector.tensor_copy(
                    nf[:].rearrange("p a b -> p (a b)"),
                    idxn[:].rearrange("p a b -> p (a b)"))
                of_neg = smp.tile([128, N_SUB, N_NEG], F32, tag='ofn')
                nc.vector.tensor_tensor(
                    out=of_neg[:], in0=nf[:],
                    in1=ks2[:].rearrange("p a -> p a ()").to_broadcast([128, N_SUB, N_NEG]),
                    op=mybir.AluOpType.add)
                oi_neg = smp.tile([128, N_SUB, N_NEG], I32, tag='oin')
                nc.vector.tensor_copy(
                    oi_neg[:].rearrange("p a b -> p (a b)"),
                    of_neg[:].rearrange("p a b -> p (a b)"))

                # phase B: element gathers of softplus tables
                scores = midp.tile([128, N_SUB, N_NEG + 1], F32, tag='sc')
                for j in range(N_SUB):
                    nc.gpsimd.indirect_dma_start(
                        out=scores[:, j, 0:1], out_offset=None, in_=gp_flat[:],
                        in_offset=bass.IndirectOffsetOnAxis(
                            ap=oi_pos[:, j:j + 1], axis=0))
                    for n in range(N_NEG):
                        nc.gpsimd.indirect_dma_start(
                            out=scores[:, j, n + 1:n + 2], out_offset=None,
                            in_=gn_flat[:],
                            in_offset=bass.IndirectOffsetOnAxis(
                                ap=oi_neg[:, j, n:n + 1], axis=0))
                csum = smp.tile([128, 1], F32, tag='csum')
                nc.vector.tensor_reduce(
                    csum[:], scores[:].rearrange("p a b -> p (a b)"),
                    axis=mybir.AxisListType.X, op=mybir.AluOpType.add)
                nc.vector.tensor_add(acc[:], acc[:], csum[:])

            nc.sync.dma_start(loss_out[:], acc[:])
    nc.finalize()
    return nc


# --------------------------------------------------------------------------- #
# host wrapper
# --------------------------------------------------------------------------- #

def kernel(w, c, neg, temp, gumbel_noise, node_emb, ctx_emb, community_w):
    w = np.ascontiguousarray(np.asarray(w, dtype=np.int64).astype(np.int32))
    c = np.ascontiguousarray(np.asarray(c, dtype=np.int64).astype(np.int32))
    neg = np.ascontiguousarray(np.asarray(neg, dtype=np.int64).astype(np.int32))
    gumbel = np.ascontiguousarray(np.asarray(gumbel_noise, dtype=np.float32))
    node = np.ascontiguousarray(np.asarray(node_emb, dtype=np.float32))
    ctx = np.ascontiguousarray(np.asarray(ctx_emb, dtype=np.float32))
    cw = np.ascontiguousarray(np.asarray(community_w, dtype=np.float32))
    tval = float(np.asarray(temp))
    assert tval > 0, "temp must be > 0 (argmax invariance)"

    rows = SIZE // N_CORES
    if 'l1' not in _COMPILED:
        _COMPILED['l1'] = build_l1(rows)
    res1 = _run_spmd(
        _COMPILED['l1'],
        [{'ctx': ctx[i * rows:(i + 1) * rows], 'cw': cw} for i in range(N_CORES)])
    gp = np.ascontiguousarray(np.concatenate([r['gp'] for r in res1.results], axis=1))
    gn = np.ascontiguousarray(np.concatenate([r['gn'] for r in res1.results], axis=1))

    # host relayout of index arrays: batch = core*B_CORE + ch*CHUNK + j*128 + p
    def relay(a):
        # a: [B] or [B, n] -> per core [N_CHUNK, 128, N_SUB(, n)]
        a2 = a.reshape(N_CORES, N_CHUNK, N_SUB, 128, *a.shape[1:])
        return np.ascontiguousarray(np.moveaxis(a2, 3, 2))

    wl, cl, negl = relay(w), relay(c), relay(neg)

    if 'l2' not in _COMPILED:
        _COMPILED['l2'] = build_l2()
    in_maps = []
    for i in range(N_CORES):
        in_maps.append({
            'node': node, 'gp': gp, 'gn': gn, 'cw': cw,
            'wl': wl[i], 'cl': cl[i], 'negl': negl[i],
            'gum': gumbel[i * B_CORE:(i + 1) * B_CORE],
        })
    res2 = _run_spmd(_COMPILED['l2'], in_maps)

    y = np.concatenate([r['y_out'] for r in res2.results], axis=0)
    prior = np.concatenate([r['p_out'] for r in res2.results], axis=0)
    loss = np.float32(sum(float(r['loss_out'].sum()) for r in res2.results) / B)
    return loss, y, prior
